# revision 36
# baseline (speedup 1.0000x reference)
"""DiT block kernel for 8 Trainium2 NeuronCores.

Sharding: data-parallel over batch (B=8 -> one batch element per core).

Vs the 856us baseline:
  - QKV + attn@V run fp8e4 DoubleRow (2 k-tiles per pass, 0.5 cyc/row):
    h1T cast fp8 (scale 1), qkv_w fp8 per-column, v_sb fp8 (xS_V),
    attn probs fp8 (as before). MLP + ada stay bf16 (fp8 there blows
    the 2e-2 error budget; measured by numpy ablation).
  - exp batched over two PSUM banks ([128,1024] per ACT instruction)
    to amortize the ~175ns fixed ACT overhead; halves instruction count.
  - fc1 bias moved into the gelu activation (per-partition bias AP);
    kills 64 ones-row bias matmuls.
  - small row copies / residual adds moved to the idle GPSIMD engine
    to unload DVE (drow eviction, v eviction, proj/fc2 residual adds).
  - x kept bf16 in SBUF (halves LN/norm DVE traffic and x DMA).
"""

import sys

sys.path.insert(0, "/opt/trn_rl_repo")

import numpy as np
import ml_dtypes

import concourse.bacc as bacc
import concourse.tile as tile
from concourse import mybir
from concourse import bass_utils
from concourse.masks import make_identity

F32 = mybir.dt.float32
BF16 = mybir.dt.bfloat16
FP8 = mybir.dt.float8e4
BF = ml_dtypes.bfloat16
F8NP = ml_dtypes.float8_e4m3
OP = mybir.AluOpType
AF = mybir.ActivationFunctionType
DR = mybir.MatmulPerfMode.DoubleRow

B = 8
L = 1024
D = 1024
H = 16
HD = 64
FF = 4096
LN_EPS = 1e-5
P = 128
TT = L // P
KT = D // P
FT = FF // P
QC = 512
NQC = L // QC

S_QK = 32.0          # q/k fp8 scale
S_V = 32.0           # v fp8 scale
ESC = 0.125 / (S_QK * S_QK)

_CACHE = {}


def _build():
    nc = bacc.Bacc(None, target_bir_lowering=False)
    names = {}
    with tile.TileContext(nc) as tc, \
            tc.tile_pool(name="dram", bufs=1, space="DRAM") as dram, \
            tc.tile_pool(name="per", bufs=1) as per:

        x_d = dram.tile([P, TT, D], BF16, kind="ExternalInput", name="x")
        c_d = dram.tile([P, KT], F32, kind="ExternalInput", name="c")
        lnT_d = dram.tile([P, 4 * KT], F32, kind="ExternalInput", name="lnT")
        adaw_d = dram.tile([P, KT, 6 * D], BF16, kind="ExternalInput",
                           name="ada_w")
        adab_d = dram.tile([1, 6 * D], BF16, kind="ExternalInput",
                           name="ada_b")
        qkvw_d = dram.tile([P, KT, 3 * D], FP8, kind="ExternalInput",
                           name="qkv_w")
        bqk_d = dram.tile([P, 16], F32, kind="ExternalInput", name="bqk")
        dqqk_d = dram.tile([P, 16], F32, kind="ExternalInput", name="dqqk")
        vbs_d = dram.tile([1, D], BF16, kind="ExternalInput", name="vbs")
        dqv_d = dram.tile([1, D], BF16, kind="ExternalInput", name="dqv")
        projw_d = dram.tile([P, KT, D], FP8, kind="ExternalInput",
                            name="proj_w")
        pbs_d = dram.tile([1, D], BF16, kind="ExternalInput", name="pbs")
        dqp_d = dram.tile([1, D], BF16, kind="ExternalInput", name="dqp")
        fc1w_d = dram.tile([P, KT, FF], BF16, kind="ExternalInput",
                           name="fc1_w")
        bf1T_d = dram.tile([P, FT], F32, kind="ExternalInput", name="bf1T")
        fc2w_d = dram.tile([P, FT, D], BF16, kind="ExternalInput",
                           name="fc2_w")
        f2bs_d = dram.tile([1, D], BF16, kind="ExternalInput", name="f2bs")
        out_d = dram.tile([L, D], F32, kind="ExternalOutput", name="out")
        for t, n in [(x_d, "x"), (c_d, "c"), (lnT_d, "lnT"),
                     (adaw_d, "ada_w"), (adab_d, "ada_b"),
                     (qkvw_d, "qkv_w"), (bqk_d, "bqk"), (dqqk_d, "dqqk"),
                     (vbs_d, "vbs"), (dqv_d, "dqv"),
                     (projw_d, "proj_w"), (pbs_d, "pbs"), (dqp_d, "dqp"),
                     (fc1w_d, "fc1_w"), (bf1T_d, "bf1T"),
                     (fc2w_d, "fc2_w"), (f2bs_d, "f2bs"), (out_d, "out")]:
            names[n] = t.name
        out_v = out_d[:].rearrange("(t p) d -> p t d", p=P)

        x_sb = per.tile([P, TT, D], BF16)
        for t in range(TT):
            eng = nc.sync if t < 4 else nc.scalar
            eng.dma_start(x_sb[:, t, :], x_d[:, t, :])
        ident = per.tile([P, P], BF16)
        make_identity(nc, ident[:])
        onesrow = per.tile([1, 512], BF16)
        nc.vector.memset(onesrow[:], 1.0)
        eps_sb = per.tile([P, 1], F32)
        nc.vector.memset(eps_sb[:], LN_EPS)

        lnT = per.tile([P, 4 * KT], F32)
        nc.sync.dma_start(lnT[:], lnT_d[:])
        bqk = per.tile([P, 16], F32)
        nc.sync.dma_start(bqk[:], bqk_d[:])
        dqqk = per.tile([P, 16], F32)
        nc.sync.dma_start(dqqk[:], dqqk_d[:])
        vbs = per.tile([1, D], BF16)
        nc.sync.dma_start(vbs[:], vbs_d[:])
        dqv_row = per.tile([1, D], BF16)
        nc.sync.dma_start(dqv_row[:], dqv_d[:])
        pbs = per.tile([1, D], BF16)
        nc.sync.dma_start(pbs[:], pbs_d[:])
        dqp_row = per.tile([1, D], BF16)
        nc.sync.dma_start(dqp_row[:], dqp_d[:])

        f2bs = per.tile([1, D], BF16)
        nc.sync.dma_start(f2bs[:], f2bs_d[:])
        bf1T = per.tile([P, FT], F32)
        nc.sync.dma_start(bf1T[:], bf1T_d[:])
        adab = per.tile([1, 6 * D], BF16)
        nc.sync.dma_start(adab[:], adab_d[:])
        projw = per.tile([P, KT, D], FP8)
        for k in range(KT):
            # ACT-engine HW DGE queue: keeps the sync queue free for the
            # startup-critical x / ada / qkv weight streams
            nc.scalar.dma_start(projw[:, k, :], projw_d[:, k, :])
        dqvb = per.tile([P, D], BF16)
        nc.gpsimd.partition_broadcast(dqvb[:], dqv_row[:])

        eff1s = per.tile([P, KT], F32)
        eff1h = per.tile([P, KT], F32)
        eff2s = per.tile([P, KT], F32)
        eff2h = per.tile([P, KT], F32)
        g1bc = per.tile([P, D], BF16)
        g2bc = per.tile([P, D], BF16)

        q8 = per.tile([P, KT, L], FP8)
        k8 = per.tile([P, KT, L], FP8)
        v_sb = per.tile([P, TT, H, HD + 1], FP8)
        nc.vector.memset(v_sb[:, :, :, HD:HD + 1], 1.0)
        h2T = per.tile([P, KT, QC], BF16)
        geluT = per.tile([P, FT, QC], BF16)

        _work_cm = tc.tile_pool(name="work", bufs=2)
        work = _work_cm.__enter__()

        def ln_stats(x_aps):
            n = len(x_aps)
            mv = work.tile([P, n, 2], F32, tag=f"mv{n}", name=f"mv{n}")
            for i, x_ap in enumerate(x_aps):
                stats = work.tile([P, 2, 6], F32, tag="stats")
                for sg in range(2):
                    nc.vector.bn_stats(stats[:, sg, :],
                                       x_ap[:, sg * 512:(sg + 1) * 512])
                nc.vector.bn_aggr(mv[:, i, :], stats[:])
            rstd = work.tile([P, n], F32, tag=f"rstd{n}", name=f"rstd{n}")
            nc.scalar.activation(rstd[:], mv[:, :, 1], AF.Sqrt, bias=eps_sb[:])
            nc.vector.reciprocal(rstd[:], rstd[:])
            return mv, rstd

        mv1, rstd1 = ln_stats([x_sb[:, t, :] for t in range(TT)])

        # ---------------- ada (bf16, 512-wide moving weights) -----------
        with nc.named_scope("ada"), \
                tc.tile_pool(name="ada_sb", bufs=1) as asb, \
                tc.tile_pool(name="ada_stream", bufs=2) as ast, \
                tc.tile_pool(name="ps_ada", bufs=2, space="PSUM") as psa, \
                tc.tile_pool(name="ps_t6", bufs=1, space="PSUM") as pst6:
            c_sb = asb.tile([P, KT], F32)
            nc.sync.dma_start(c_sb[:], c_d[:])
            silu_b = asb.tile([P, KT], BF16)
            nc.scalar.activation(silu_b[:], c_sb[:], AF.Silu)
            ssgb = asb.tile([1, 6 * D], BF16, name="ssgb")

            def ada_chunk(nch, eng=None):
                aw = ast.tile([P, KT, 512], BF16, tag="aw")
                (eng or nc.sync).dma_start(
                    aw[:], adaw_d[:, :, nch * 512:(nch + 1) * 512])
                pa = psa.tile([1, 512], F32, tag="ada")
                nc.tensor.matmul(pa[:], onesrow[:, 0:1],
                                 adab[:, nch * 512:(nch + 1) * 512],
                                 start=True, stop=False)
                for k in range(KT):
                    nc.tensor.matmul(pa[:], silu_b[:, k:k + 1], aw[:, k, :],
                                     start=False, stop=(k == KT - 1),
                                     skip_group_check=True)
                nc.vector.tensor_copy(
                    ssgb[:, nch * 512:(nch + 1) * 512], pa[:])

            # shift/scale rows -> feature-major cols; v' = sh1,sc1,sh2,sc2
            pt6 = pst6.tile([P, 32, 2], BF16)

            def ada_tr(vi, v):
                for ch in range(KT):
                    i = ch * 4 + vi
                    nc.tensor.transpose(
                        pt6[:, i, 0:1],
                        ssgb[0:1, v * D + ch * P:v * D + (ch + 1) * P],
                        ident[0:1, 0:1])
                nc.vector.tensor_copy(ssgT[:, :, vi], pt6[:, vi:32:4, 0])

            ssgT = asb.tile([P, KT, 4], F32, name="ssgT")
            t0 = asb.tile([P, KT], F32, name="t0")
            t1 = asb.tile([P, KT], F32, name="t1")
            # chunk order puts the attn shift/scale (-> h1T -> qkv) first
            for nch in (0, 1, 2, 3):
                ada_chunk(nch)
            ada_tr(0, 0)
            ada_tr(1, 1)
            nc.vector.tensor_scalar_add(t0[:], ssgT[:, :, 1], 1.0)
            nc.vector.tensor_mul(eff1s[:], t0[:], lnT[:, 0:KT])
            nc.vector.tensor_mul(t1[:], t0[:], lnT[:, KT:2 * KT])
            nc.vector.tensor_add(eff1h[:], t1[:], ssgT[:, :, 0])
            for nch in (6, 7, 8, 9):
                ada_chunk(nch, nc.scalar)
            ada_tr(2, 3)
            ada_tr(3, 4)
            nc.vector.tensor_scalar_add(t0[:], ssgT[:, :, 3], 1.0)
            nc.vector.tensor_mul(eff2s[:], t0[:], lnT[:, 2 * KT:3 * KT])
            nc.vector.tensor_mul(t1[:], t0[:], lnT[:, 3 * KT:4 * KT])
            nc.vector.tensor_add(eff2h[:], t1[:], ssgT[:, :, 2])
            for nch in (4, 5, 10, 11):
                ada_chunk(nch, nc.scalar)
            g1row = asb.tile([1, D], BF16, name="g1row")
            nc.vector.tensor_mul(g1row[:], ssgb[:, 2 * D:3 * D], dqp_row[:])
            nc.gpsimd.partition_broadcast(g1bc[:], g1row[:])
            nc.gpsimd.partition_broadcast(g2bc[:], ssgb[:, 5 * D:6 * D])

        # ---------------- LN -> hT (transpose + fused modulate) ---------
        def build_hT(dst, eff_s, eff_h, mv, rstd, idx0, tg, toff, pstr):
            norms = []
            for s in range(4):
                t = toff + s
                nrm = work.tile([P, D], BF16, tag="nrm", bufs=4)
                i = idx0 + s
                nc.vector.tensor_scalar(out=nrm[:], in0=x_sb[:, t, :],
                                        scalar1=mv[:, i, 0:1],
                                        scalar2=rstd[:, i:i + 1],
                                        op0=OP.subtract, op1=OP.mult)
                norms.append(nrm)
            for k in range(KT):
                pt = pstr.tile([P, 4 * P], BF16, tag="tr")
                for s in range(4):
                    nc.tensor.transpose(pt[:, s * P:(s + 1) * P],
                                        norms[s][:, k * P:(k + 1) * P],
                                        ident[:])
                nc.vector.tensor_scalar(
                    out=dst[:, k, tg * 512:(tg + 1) * 512], in0=pt[:],
                    scalar1=eff_s[:, k:k + 1], scalar2=eff_h[:, k:k + 1],
                    op0=OP.mult, op1=OP.add)

        # ---------------- prologue: h1T + QKV (fp8 DoubleRow) -----------
        pro_d = {}
        wv_half = []

        def open_prologue():
            pro_d["pro_cm"] = tc.tile_pool(name="pro", bufs=1)
            pro = pro_d["pro_cm"].__enter__()
            h1T = pro.tile([P, KT, L], FP8, name="h1T")
            with tc.tile_pool(name="ps_tr1", bufs=1, space="PSUM") as pstr1:
                for tg in range(2):
                    build_hT(h1T, eff1s, eff1h, mv1, rstd1, tg * 4, tg,
                             tg * 4, pstr1)
            pro_d["qs_cm"] = tc.tile_pool(name="qkv_stream", bufs=3)
            pro_d["qst"] = pro_d["qs_cm"].__enter__()
            pro_d["h1T"] = h1T

        def qk_pair(qk, jp):
            fbase = D if qk == 0 else 0
            dst = k8 if qk == 0 else q8
            wj = pro_d["qst"].tile([P, KT, 2 * P], FP8, tag="wj", bufs=2,
                                   name="wjt")
            nc.sync.dma_start(
                wj[:], qkvw_d[:, :, fbase + jp * P:fbase + (jp + 2) * P])
            for jo in range(2):
                jj = jp + jo
                ci = (fbase + jj * P) // P
                for tg in range(2):
                    pq = mm_tile(jj * 2 + tg, "pq")
                    for k in range(0, KT, 2):
                        nc.tensor.matmul(
                            pq[:], wj[:, k:k + 2, jo * P:(jo + 1) * P],
                            pro_d["h1T"][:, k:k + 2,
                                         tg * 512:(tg + 1) * 512],
                            start=(k == 0), stop=(k == KT - 2),
                            perf_mode=DR)
                    nc.vector.tensor_scalar(
                        out=dst[:, jj, tg * 512:(tg + 1) * 512], in0=pq[:],
                        scalar1=dqqk[:, ci:ci + 1], scalar2=bqk[:, ci:ci + 1],
                        op0=OP.mult, op1=OP.add)

        def v_chunk(tt, fh):
            pv = mm_tile(tt * 2 + fh, "pv")
            nc.tensor.matmul(pv[:], onesrow[:, 0:P],
                             vbs[:, fh * 512:(fh + 1) * 512],
                             start=True, stop=False)
            for k in range(0, KT, 2):
                nc.tensor.matmul(pv[:],
                                 pro_d["h1T"][:, k:k + 2, tt * P:(tt + 1) * P],
                                 wv_half[fh][:, k:k + 2, :],
                                 start=False, stop=(k == KT - 2),
                                 skip_group_check=True, perf_mode=DR)
            nc.vector.tensor_tensor(
                out=v_sb[:, tt, fh * 8:(fh + 1) * 8, 0:HD], in0=pv[:],
                in1=dqvb[:, fh * 512:(fh + 1) * 512], op=OP.mult)

        def load_wv(fh):
            wv = pro_d["qst"].tile([P, KT, 512], FP8, tag="wv", bufs=1,
                          name=f"wv{fh}")
            nc.sync.dma_start(wv[:],
                              qkvw_d[:, :, 2 * D + fh * 512:
                                     2 * D + (fh + 1) * 512])
            wv_half.append(wv)

        # ---------------- attention + MLP, phase-pipelined --------------
        fsd = {}

        def fs_tile(*a, **k):
            return fsd["fs"].tile(*a, **k)

        with tc.tile_pool(name="attn", bufs=2) as ap, \
                tc.tile_pool(name="aTp", bufs=1) as aTp, \
                tc.tile_pool(name="ps_m", bufs=1, space="PSUM") as psm:

            attn_ps = {}
            attn_ps["pss_cm"] = tc.tile_pool(name="ps_s", bufs=2,
                                             space="PSUM")
            pss = attn_ps["pss_cm"].__enter__()
            attn_ps["psav_cm"] = tc.tile_pool(name="ps_av", bufs=1,
                                              space="PSUM")
            psav = attn_ps["psav_cm"].__enter__()

            def mm_tile(i, nm):
                return psm.tile([P, 512], F32, tag=f"f2_{i % 2}", bufs=1,
                                name=nm)

            open_prologue()

            def load_v(fh):
                load_wv(fh)
                for tt in range(TT):
                    v_chunk(tt, fh)

            def scores_exp(qc, j):
                q0 = qc * QC
                attA = ap.tile([P, TT, QC], FP8, tag="attA")
                attB = ap.tile([P, TT, QC], FP8, tag="attB")
                for hh, att in ((0, attA), (1, attB)):
                    p0 = hh * HD
                    for mp in range(TT // 2):
                        ps2 = pss.tile([P, 2, QC], F32, tag="sc", name="ps2")
                        for s in range(2):
                            m = 2 * mp + s
                            nc.tensor.matmul(
                                ps2[:, s, :],
                                k8[p0:p0 + HD, j, m * P:(m + 1) * P],
                                q8[p0:p0 + HD, j, q0:q0 + QC],
                                start=True, stop=True,
                                tile_position=(p0, 0))
                        nc.scalar.activation(att[:, 2 * mp:2 * mp + 2, :],
                                             ps2[:], AF.Exp, scale=ESC)
                return attA, attB

            def av_norm(j, attA, attB, aT):
                for hh, att in ((0, attA), (1, attB)):
                    h = 2 * j + hh
                    pu = psav.tile([HD + 1, QC], F32, tag="pu")
                    for mp in range(TT // 2):
                        nc.tensor.matmul(
                            pu[:], v_sb[:, 2 * mp:2 * mp + 2, h, :],
                            att[:, 2 * mp:2 * mp + 2, :],
                            start=(mp == 0), stop=(mp == TT // 2 - 1),
                            perf_mode=DR)
                    drow = work.tile([1, QC], F32, tag="drow", bufs=1)
                    nc.vector.tensor_copy(drow[:], pu[HD:HD + 1, :])
                    dbc = work.tile([HD, QC], F32, tag="dbc", bufs=1)
                    nc.gpsimd.partition_broadcast(dbc[:], drow[:])
                    rec = work.tile([HD, QC], F32, tag="rec", bufs=1)
                    nc.vector.reciprocal_approx_fast(rec[:], dbc[:])
                    nc.vector.tensor_tensor(
                        out=aT[hh * 64:(hh + 1) * 64, j, :],
                        in0=pu[0:HD, :], in1=rec[:], op=OP.mult)

            def proj_block(qc, aT, s):
                t_global = qc * 4 + s
                for fh in range(2):
                    pp = mm_tile(s * 2 + fh, "pp")
                    nc.tensor.matmul(pp[:], onesrow[:, 0:P],
                                     pbs[:, fh * 512:(fh + 1) * 512],
                                     start=True, stop=False)
                    for k in range(KT):
                        nc.tensor.matmul(
                            pp[:], aT[:, k, s * P:(s + 1) * P],
                            projw[:, k, fh * 512:(fh + 1) * 512],
                            start=False, stop=(k == KT - 1),
                            skip_group_check=True)
                    tmp = work.tile([P, 512], BF16, tag="ptmp", bufs=1)
                    nc.vector.tensor_tensor(
                        out=tmp[:], in0=pp[:],
                        in1=g1bc[:, fh * 512:(fh + 1) * 512], op=OP.mult)
                    nc.gpsimd.tensor_add(
                        x_sb[:, t_global, fh * 512:(fh + 1) * 512],
                        x_sb[:, t_global, fh * 512:(fh + 1) * 512],
                        tmp[:])

            def fc1_gelu(qc):
                # 4-bank pf rotation: PE stays 4 gelu-evictions ahead so
                # the scheduler never slots attn exps between gelus (which
                # would thrash the ACT table)
                for fcp in range(0, FT, 4):
                    w1 = fs_tile([P, KT, 4 * P], BF16, tag="w1", bufs=3,
                                 name="w1t")
                    nc.sync.dma_start(w1[:],
                                      fc1w_d[:, :, fcp * P:(fcp + 4) * P])
                    if qc == 0:
                        sc = pss.tile([P, 2, QC], F32, tag="sc",
                                      name="pfsc")
                        extra = [sc[:, 0, :], sc[:, 1, :]]
                    else:
                        tl0 = fsd["tail"].tile([P, 512], F32, tag="tl0",
                                               bufs=1, name="pft0")
                        tl1 = fsd["tail"].tile([P, 512], F32, tag="tl1",
                                               bufs=1, name="pft1")
                        extra = [tl0[:], tl1[:]]
                    pfs = [mm_tile(0, "pf")[:], mm_tile(1, "pf")[:],
                           extra[0], extra[1]]
                    for i in range(4):
                        fc = fcp + i
                        pf = pfs[i]
                        for k in range(KT):
                            nc.tensor.matmul(pf,
                                             w1[:, k, i * P:(i + 1) * P],
                                             h2T[:, k, :],
                                             start=(k == 0),
                                             stop=(k == KT - 1))
                        nc.scalar.activation(geluT[:, fc, :], pf,
                                             AF.Gelu_apprx_tanh,
                                             bias=bf1T[:, fc:fc + 1])

            def fc2_blocks(qc):
                # s-pairs share the streamed w2 chunk; 2 psum banks live
                for sp in range(2):
                    for fp in range(2):
                        def blk(sp=sp, fp=fp, qc=qc):
                            ps2 = [mm_tile(0, "pf2a"), mm_tile(1, "pf2b")]
                            for s in range(2):
                                nc.tensor.matmul(
                                    ps2[s][:], onesrow[:, 0:P],
                                    f2bs[:, fp * 512:(fp + 1) * 512],
                                    start=True, stop=False)
                            for fpr in range(FT // 2):
                                w2 = fs_tile([P, 2, 512], BF16, tag="w2",
                                             bufs=3, name="w2t")
                                nc.sync.dma_start(
                                    w2[:], fc2w_d[:, 2 * fpr:2 * fpr + 2,
                                                  fp * 512:(fp + 1) * 512])
                                for i in range(2):
                                    ft = 2 * fpr + i
                                    for s in range(2):
                                        tok = sp * 2 + s
                                        nc.tensor.matmul(
                                            ps2[s][:],
                                            geluT[:, ft,
                                                  tok * P:(tok + 1) * P],
                                            w2[:, i, :],
                                            start=False,
                                            stop=(ft == FT - 1),
                                            skip_group_check=True)
                            for s in range(2):
                                t_global = qc * 4 + sp * 2 + s
                                tmp = work.tile([P, 512], BF16, tag="ftmp",
                                                bufs=1)
                                nc.vector.tensor_tensor(
                                    out=tmp[:], in0=ps2[s][:],
                                    in1=g2bc[:, fp * 512:(fp + 1) * 512],
                                    op=OP.mult)
                                oth = work.tile([P, 512], F32, tag="ot",
                                                bufs=2,
                                                name=f"ot{qc}{fp}{sp}{s}")
                                nc.gpsimd.tensor_add(
                                    oth[:],
                                    x_sb[:, t_global,
                                         fp * 512:(fp + 1) * 512],
                                    tmp[:])
                                nc.scalar.dma_start(
                                    out_v[:, t_global,
                                          fp * 512:(fp + 1) * 512],
                                    oth[:])
                        yield blk

            def fc2_tail_blocks(qc):
                # post-attn: stream each w2 chunk ONCE for all 4 token
                # blocks (4 live psum banks) -- halves fc2 HBM traffic
                for fp in range(2):
                    def blk(fp=fp, qc=qc):
                        tl = fsd["tail"]
                        ps4 = [tl.tile([P, 512], F32, tag=f"tl{i}", bufs=1,
                                       name=f"ptl{i}") for i in range(4)]
                        for s in range(4):
                            nc.tensor.matmul(
                                ps4[s][:], onesrow[:, 0:P],
                                f2bs[:, fp * 512:(fp + 1) * 512],
                                start=True, stop=False)
                        for fpr in range(FT // 2):
                            w2 = fs_tile([P, 2, 512], BF16, tag="w2",
                                         bufs=3, name="w2t")
                            nc.sync.dma_start(
                                w2[:], fc2w_d[:, 2 * fpr:2 * fpr + 2,
                                              fp * 512:(fp + 1) * 512])
                            for i in range(2):
                                ft = 2 * fpr + i
                                for s in range(4):
                                    nc.tensor.matmul(
                                        ps4[s][:],
                                        geluT[:, ft, s * P:(s + 1) * P],
                                        w2[:, i, :],
                                        start=False, stop=(ft == FT - 1),
                                        skip_group_check=True)
                        for s in range(4):
                            t_global = qc * 4 + s
                            tmp = work.tile([P, 512], BF16, tag="ftmp",
                                            bufs=1)
                            nc.vector.tensor_tensor(
                                out=tmp[:], in0=ps4[s][:],
                                in1=g2bc[:, fp * 512:(fp + 1) * 512],
                                op=OP.mult)
                            oth = work.tile([P, 512], F32, tag="ot",
                                            bufs=2, name=f"otl{qc}{fp}{s}")
                            nc.gpsimd.tensor_add(
                                oth[:],
                                x_sb[:, t_global, fp * 512:(fp + 1) * 512],
                                tmp[:])
                            nc.scalar.dma_start(
                                out_v[:, t_global, fp * 512:(fp + 1) * 512],
                                oth[:])
                    yield blk

            closed = {}

            def run_qc(qc, pending, producers=None):
                pend_i = 0
                with nc.named_scope(f"attn{qc}"):
                    aT = aTp.tile([P, KT, QC], BF16, tag="aT",
                                  name=f"aT_{qc}")
                    atts = []
                    for j in range(KT):
                        if producers:
                            for fn in producers.pop(j, []):
                                fn()
                        atts.append(scores_exp(qc, j))
                        if j % 2 == 0 and pend_i < len(pending):
                            pending[pend_i]()
                            pend_i += 1
                        if j >= 1:
                            av_norm(j - 1, *atts[j - 1], aT)
                    av_norm(KT - 1, *atts[KT - 1], aT)
                if qc == 0 and not closed:
                    # h1T / V weights are dead: free prologue SBUF
                    pro_d["qs_cm"].__exit__(None, None, None)
                    pro_d["pro_cm"].__exit__(None, None, None)
                    fsd["cm"] = tc.tile_pool(name="fc_stream", bufs=2)
                    fsd["fs"] = fsd["cm"].__enter__()
                    closed["done"] = True
                if qc == 1:
                    # scores/av psum dead: free 5 banks for the 4-bank tail
                    attn_ps["psav_cm"].__exit__(None, None, None)
                    attn_ps["pss_cm"].__exit__(None, None, None)
                    fsd["tail_cm"] = tc.tile_pool(name="ps_tail", bufs=1,
                                                  space="PSUM")
                    fsd["tail"] = fsd["tail_cm"].__enter__()
                with nc.named_scope(f"proj{qc}"):
                    mv2 = work.tile([P, 4, 2], F32, tag="mv4", name="mv4")
                    for s in range(4):
                        proj_block(qc, aT, s)
                        # ln2 stats for this tile right away (fills the
                        # proj->ln2 dependency valley)
                        stats = work.tile([P, 2, 6], F32, tag="stats")
                        x_ap = x_sb[:, qc * 4 + s, :]
                        for sg in range(2):
                            nc.vector.bn_stats(
                                stats[:, sg, :],
                                x_ap[:, sg * 512:(sg + 1) * 512])
                        nc.vector.bn_aggr(mv2[:, s, :], stats[:])
                with nc.named_scope(f"ln2_{qc}"):
                    rstd2 = work.tile([P, 4], F32, tag="rstd4",
                                      name="rstd4")
                    nc.scalar.activation(rstd2[:], mv2[:, :, 1], AF.Sqrt,
                                         bias=eps_sb[:])
                    nc.vector.reciprocal(rstd2[:], rstd2[:])
                    with tc.tile_pool(name="ps_tr2", bufs=1,
                                      space="PSUM") as pstr2:
                        build_hT(h2T, eff2s, eff2h, mv2, rstd2, 0, 0,
                                 qc * 4, pstr2)
                with nc.named_scope(f"fc1_{qc}"):
                    fc1_gelu(qc)
                    while pend_i < len(pending):
                        pending[pend_i]()
                        pend_i += 1
                if qc == 1:
                    return list(fc2_tail_blocks(qc))
                return list(fc2_blocks(qc))

            producers0 = {
                0: [lambda: qk_pair(0, 0), lambda: qk_pair(1, 0),
                    lambda: load_v(0)],
                1: [lambda: qk_pair(0, 2), lambda: qk_pair(1, 2)],
                2: [lambda: qk_pair(0, 4), lambda: qk_pair(1, 4),
                    lambda: load_v(1)],
                3: [lambda: qk_pair(0, 6), lambda: qk_pair(1, 6)],
            }
            pending = run_qc(0, [], producers0)
            pending = run_qc(1, pending)
            with nc.named_scope("mlp_tail"):
                for blk in pending:
                    blk()
            fsd["tail_cm"].__exit__(None, None, None)
            fsd["cm"].__exit__(None, None, None)

        _work_cm.__exit__(None, None, None)

    nc.compile()
    return nc, names


def _get_compiled():
    if "nc" not in _CACHE:
        _CACHE["nc"], _CACHE["names"] = _build()
    return _CACHE["nc"], _CACHE["names"]


def _q8col(w, smax=224.0):
    w = np.asarray(w, np.float32)
    am = np.abs(w).max(axis=0, keepdims=True)
    s = np.where(am > 0, smax / np.maximum(am, 1e-30), 1.0)
    w8 = np.clip(w * s, -240, 240).astype(F8NP)
    return w8, s[0]


def _pmajor(w):
    w = np.asarray(w)
    kp, n = w.shape
    return np.ascontiguousarray(w.reshape(kp // P, P, n).transpose(1, 0, 2))


def _prep_maps(names, x, c, ln1_w, ln1_b, ln2_w, ln2_b, ada_w, ada_b,
               qkv_w, qkv_b, proj_w, proj_b, fc1_w, fc1_b, fc2_w, fc2_b):
    x = np.asarray(x, np.float32)
    c = np.asarray(c, np.float32)
    qkv8, s_qkv = _q8col(qkv_w)
    proj8, s_proj = _q8col(proj_w)
    qkv_b = np.asarray(qkv_b, np.float32)
    s_wv = s_qkv[2 * D:]

    def tcols(v):
        return np.asarray(v, np.float32).reshape(KT, P).T
    lnT = np.concatenate([tcols(ln1_w), tcols(ln1_b),
                          tcols(ln2_w), tcols(ln2_b)], axis=1)

    common = {
        names["lnT"]: np.ascontiguousarray(lnT, np.float32),
        names["ada_w"]: _pmajor(ada_w).astype(BF),
        names["ada_b"]: np.asarray(ada_b).astype(BF).reshape(1, -1),
        names["qkv_w"]: _pmajor(qkv8),
        names["bqk"]: np.ascontiguousarray(
            (qkv_b[0:2 * D] * S_QK).reshape(16, P).T.astype(np.float32)),
        names["dqqk"]: np.ascontiguousarray(
            (S_QK / s_qkv[0:2 * D]).reshape(16, P).T.astype(np.float32)),
        names["vbs"]: (qkv_b[2 * D:] * s_wv).astype(BF).reshape(1, D),
        names["dqv"]: (S_V / s_wv).astype(BF).reshape(1, D),
        names["proj_w"]: _pmajor(proj8),
        names["pbs"]: (np.asarray(proj_b, np.float32) * S_V * s_proj)
        .astype(BF).reshape(1, D),
        names["dqp"]: (1.0 / (S_V * s_proj)).astype(BF).reshape(1, D),
        names["fc1_w"]: _pmajor(fc1_w).astype(BF),
        names["bf1T"]: np.ascontiguousarray(
            np.asarray(fc1_b, np.float32).reshape(FT, P).T),
        names["fc2_w"]: _pmajor(fc2_w).astype(BF),
        names["f2bs"]: np.asarray(fc2_b).astype(BF).reshape(1, D),
    }
    in_maps = []
    for b in range(B):
        m = dict(common)
        m[names["x"]] = np.ascontiguousarray(
            x[b].reshape(TT, P, D).transpose(1, 0, 2)).astype(BF)
        m[names["c"]] = np.ascontiguousarray(c[b].reshape(KT, P).T)
        in_maps.append(m)
    return in_maps


def kernel(x, c, ln1_w, ln1_b, ln2_w, ln2_b, ada_w, ada_b,
           qkv_w, qkv_b, proj_w, proj_b, fc1_w, fc1_b, fc2_w, fc2_b,
           _trace=False):
    nc, names = _get_compiled()
    in_maps = _prep_maps(names, x, c, ln1_w, ln1_b, ln2_w, ln2_b,
                         ada_w, ada_b, qkv_w, qkv_b, proj_w, proj_b,
                         fc1_w, fc1_b, fc2_w, fc2_b)
    res = bass_utils.run_bass_kernel_spmd(nc, in_maps, core_ids=list(range(B)),
                                          trace=_trace)
    out = np.stack([res.results[b][names["out"]] for b in range(B)])
    if _trace:
        _CACHE["last_result"] = res
    return out


# revision 40
# speedup vs baseline: 1.0111x; 1.0111x over previous
"""DiT block kernel for 8 Trainium2 NeuronCores.

Sharding: data-parallel over batch (B=8 -> one batch element per core).

Vs the 856us baseline:
  - QKV + attn@V run fp8e4 DoubleRow (2 k-tiles per pass, 0.5 cyc/row):
    h1T cast fp8 (scale 1), qkv_w fp8 per-column, v_sb fp8 (xS_V),
    attn probs fp8 (as before). MLP + ada stay bf16 (fp8 there blows
    the 2e-2 error budget; measured by numpy ablation).
  - exp batched over two PSUM banks ([128,1024] per ACT instruction)
    to amortize the ~175ns fixed ACT overhead; halves instruction count.
  - fc1 bias moved into the gelu activation (per-partition bias AP);
    kills 64 ones-row bias matmuls.
  - small row copies / residual adds moved to the idle GPSIMD engine
    to unload DVE (drow eviction, v eviction, proj/fc2 residual adds).
  - x kept bf16 in SBUF (halves LN/norm DVE traffic and x DMA).
"""

import sys

sys.path.insert(0, "/opt/trn_rl_repo")

import numpy as np
import ml_dtypes

import concourse.bacc as bacc
import concourse.tile as tile
from concourse import mybir
from concourse import bass_utils
from concourse.masks import make_identity

F32 = mybir.dt.float32
BF16 = mybir.dt.bfloat16
FP8 = mybir.dt.float8e4
BF = ml_dtypes.bfloat16
F8NP = ml_dtypes.float8_e4m3
OP = mybir.AluOpType
AF = mybir.ActivationFunctionType
DR = mybir.MatmulPerfMode.DoubleRow

B = 8
L = 1024
D = 1024
H = 16
HD = 64
FF = 4096
LN_EPS = 1e-5
P = 128
TT = L // P
KT = D // P
FT = FF // P
QC = 512
NQC = L // QC

S_QK = 32.0          # q/k fp8 scale
S_V = 32.0           # v fp8 scale
ESC = 0.125 / (S_QK * S_QK)

_CACHE = {}


def _build():
    nc = bacc.Bacc(None, target_bir_lowering=False)
    names = {}
    with tile.TileContext(nc) as tc, \
            tc.tile_pool(name="dram", bufs=1, space="DRAM") as dram, \
            tc.tile_pool(name="per", bufs=1) as per:

        x_d = dram.tile([P, TT, D], BF16, kind="ExternalInput", name="x")
        c_d = dram.tile([P, KT], F32, kind="ExternalInput", name="c")
        lnT_d = dram.tile([P, 4 * KT], F32, kind="ExternalInput", name="lnT")
        adaw_d = dram.tile([P, KT, 6 * D], BF16, kind="ExternalInput",
                           name="ada_w")
        adab_d = dram.tile([1, 6 * D], BF16, kind="ExternalInput",
                           name="ada_b")
        qkvw_d = dram.tile([P, KT, 3 * D], FP8, kind="ExternalInput",
                           name="qkv_w")
        bqk_d = dram.tile([P, 16], F32, kind="ExternalInput", name="bqk")
        dqqk_d = dram.tile([P, 16], F32, kind="ExternalInput", name="dqqk")
        vbs_d = dram.tile([1, D], BF16, kind="ExternalInput", name="vbs")
        dqv_d = dram.tile([1, D], BF16, kind="ExternalInput", name="dqv")
        projw_d = dram.tile([P, KT, D], FP8, kind="ExternalInput",
                            name="proj_w")
        pbs_d = dram.tile([1, D], BF16, kind="ExternalInput", name="pbs")
        dqp_d = dram.tile([1, D], BF16, kind="ExternalInput", name="dqp")
        fc1w_d = dram.tile([P, KT, FF], BF16, kind="ExternalInput",
                           name="fc1_w")
        bf1T_d = dram.tile([P, FT], F32, kind="ExternalInput", name="bf1T")
        fc2w_d = dram.tile([P, FT, D], BF16, kind="ExternalInput",
                           name="fc2_w")
        f2bs_d = dram.tile([1, D], BF16, kind="ExternalInput", name="f2bs")
        out_d = dram.tile([L, D], F32, kind="ExternalOutput", name="out")
        for t, n in [(x_d, "x"), (c_d, "c"), (lnT_d, "lnT"),
                     (adaw_d, "ada_w"), (adab_d, "ada_b"),
                     (qkvw_d, "qkv_w"), (bqk_d, "bqk"), (dqqk_d, "dqqk"),
                     (vbs_d, "vbs"), (dqv_d, "dqv"),
                     (projw_d, "proj_w"), (pbs_d, "pbs"), (dqp_d, "dqp"),
                     (fc1w_d, "fc1_w"), (bf1T_d, "bf1T"),
                     (fc2w_d, "fc2_w"), (f2bs_d, "f2bs"), (out_d, "out")]:
            names[n] = t.name
        out_v = out_d[:].rearrange("(t p) d -> p t d", p=P)

        x_sb = per.tile([P, TT, D], BF16)
        for t in range(TT):
            eng = nc.sync if t < 4 else nc.scalar
            eng.dma_start(x_sb[:, t, :], x_d[:, t, :])
        ident = per.tile([P, P], BF16)
        make_identity(nc, ident[:])
        onesrow = per.tile([1, 512], BF16)
        nc.vector.memset(onesrow[:], 1.0)
        eps_sb = per.tile([P, 1], F32)
        nc.vector.memset(eps_sb[:], LN_EPS)

        lnT = per.tile([P, 4 * KT], F32)
        nc.sync.dma_start(lnT[:], lnT_d[:])
        bqk = per.tile([P, 16], F32)
        nc.sync.dma_start(bqk[:], bqk_d[:])
        dqqk = per.tile([P, 16], F32)
        nc.sync.dma_start(dqqk[:], dqqk_d[:])
        vbs = per.tile([1, D], BF16)
        nc.sync.dma_start(vbs[:], vbs_d[:])
        dqv_row = per.tile([1, D], BF16)
        nc.sync.dma_start(dqv_row[:], dqv_d[:])
        pbs = per.tile([1, D], BF16)
        nc.sync.dma_start(pbs[:], pbs_d[:])
        dqp_row = per.tile([1, D], BF16)
        nc.sync.dma_start(dqp_row[:], dqp_d[:])

        f2bs = per.tile([1, D], BF16)
        nc.sync.dma_start(f2bs[:], f2bs_d[:])
        bf1T = per.tile([P, FT], F32)
        nc.sync.dma_start(bf1T[:], bf1T_d[:])
        adab = per.tile([1, 6 * D], BF16)
        nc.sync.dma_start(adab[:], adab_d[:])
        projw = per.tile([P, KT, D], FP8)
        for k in range(KT):
            # ACT-engine HW DGE queue: keeps the sync queue free for the
            # startup-critical x / ada / qkv weight streams
            nc.scalar.dma_start(projw[:, k, :], projw_d[:, k, :])
        dqvb = per.tile([P, D], BF16)
        nc.gpsimd.partition_broadcast(dqvb[:], dqv_row[:])

        eff1s = per.tile([P, KT], F32)
        eff1h = per.tile([P, KT], F32)
        eff2s = per.tile([P, KT], F32)
        eff2h = per.tile([P, KT], F32)
        g1bc = per.tile([P, D], BF16)
        g2bc = per.tile([P, D], BF16)

        q8 = per.tile([P, KT, L], FP8)
        k8 = per.tile([P, KT, L], FP8)
        v_sb = per.tile([P, TT, H, HD + 1], FP8)
        nc.vector.memset(v_sb[:, :, :, HD:HD + 1], 1.0)
        h2T = per.tile([P, KT, QC], BF16)
        geluT = per.tile([P, FT, QC], BF16)

        _work_cm = tc.tile_pool(name="work", bufs=2)
        work = _work_cm.__enter__()

        def ln_stats(x_aps):
            n = len(x_aps)
            mv = work.tile([P, n, 2], F32, tag=f"mv{n}", name=f"mv{n}")
            for i, x_ap in enumerate(x_aps):
                stats = work.tile([P, 2, 6], F32, tag="stats")
                for sg in range(2):
                    nc.vector.bn_stats(stats[:, sg, :],
                                       x_ap[:, sg * 512:(sg + 1) * 512])
                nc.vector.bn_aggr(mv[:, i, :], stats[:])
            rstd = work.tile([P, n], F32, tag=f"rstd{n}", name=f"rstd{n}")
            nc.scalar.activation(rstd[:], mv[:, :, 1], AF.Sqrt, bias=eps_sb[:])
            nc.vector.reciprocal(rstd[:], rstd[:])
            return mv, rstd

        mv1, rstd1 = ln_stats([x_sb[:, t, :] for t in range(TT)])

        # ---------------- ada (bf16, 512-wide moving weights) -----------
        with nc.named_scope("ada"), \
                tc.tile_pool(name="ada_sb", bufs=1) as asb, \
                tc.tile_pool(name="ada_stream", bufs=2) as ast, \
                tc.tile_pool(name="ps_ada", bufs=2, space="PSUM") as psa, \
                tc.tile_pool(name="ps_t6", bufs=1, space="PSUM") as pst6:
            c_sb = asb.tile([P, KT], F32)
            nc.sync.dma_start(c_sb[:], c_d[:])
            silu_b = asb.tile([P, KT], BF16)
            nc.scalar.activation(silu_b[:], c_sb[:], AF.Silu)
            ssgb = asb.tile([1, 6 * D], BF16, name="ssgb")

            def ada_chunk(nch, eng=None):
                aw = ast.tile([P, KT, 512], BF16, tag="aw")
                (eng or nc.sync).dma_start(
                    aw[:], adaw_d[:, :, nch * 512:(nch + 1) * 512])
                pa = psa.tile([1, 512], F32, tag="ada")
                nc.tensor.matmul(pa[:], onesrow[:, 0:1],
                                 adab[:, nch * 512:(nch + 1) * 512],
                                 start=True, stop=False)
                for k in range(KT):
                    nc.tensor.matmul(pa[:], silu_b[:, k:k + 1], aw[:, k, :],
                                     start=False, stop=(k == KT - 1),
                                     skip_group_check=True)
                nc.vector.tensor_copy(
                    ssgb[:, nch * 512:(nch + 1) * 512], pa[:])

            # shift/scale rows -> feature-major cols; v' = sh1,sc1,sh2,sc2
            pt6 = pst6.tile([P, 32, 2], BF16)

            def ada_tr(vi, v):
                for ch in range(KT):
                    i = ch * 4 + vi
                    nc.tensor.transpose(
                        pt6[:, i, 0:1],
                        ssgb[0:1, v * D + ch * P:v * D + (ch + 1) * P],
                        ident[0:1, 0:1])
                nc.vector.tensor_copy(ssgT[:, :, vi], pt6[:, vi:32:4, 0])

            ssgT = asb.tile([P, KT, 4], F32, name="ssgT")
            t0 = asb.tile([P, KT], F32, name="t0")
            t1 = asb.tile([P, KT], F32, name="t1")
            # chunk order puts the attn shift/scale (-> h1T -> qkv) first
            for nch in (0, 1, 2, 3):
                ada_chunk(nch)
            ada_tr(0, 0)
            ada_tr(1, 1)
            nc.vector.tensor_scalar_add(t0[:], ssgT[:, :, 1], 1.0)
            nc.vector.tensor_mul(eff1s[:], t0[:], lnT[:, 0:KT])
            nc.vector.tensor_mul(t1[:], t0[:], lnT[:, KT:2 * KT])
            nc.vector.tensor_add(eff1h[:], t1[:], ssgT[:, :, 0])
            for nch in (6, 7, 8, 9):
                ada_chunk(nch)
            ada_tr(2, 3)
            ada_tr(3, 4)
            nc.vector.tensor_scalar_add(t0[:], ssgT[:, :, 3], 1.0)
            nc.vector.tensor_mul(eff2s[:], t0[:], lnT[:, 2 * KT:3 * KT])
            nc.vector.tensor_mul(t1[:], t0[:], lnT[:, 3 * KT:4 * KT])
            nc.vector.tensor_add(eff2h[:], t1[:], ssgT[:, :, 2])
            for nch in (4, 5, 10, 11):
                ada_chunk(nch)
            g1row = asb.tile([1, D], BF16, name="g1row")
            nc.vector.tensor_mul(g1row[:], ssgb[:, 2 * D:3 * D], dqp_row[:])
            nc.gpsimd.partition_broadcast(g1bc[:], g1row[:])
            nc.gpsimd.partition_broadcast(g2bc[:], ssgb[:, 5 * D:6 * D])

        # ---------------- LN -> hT (transpose + fused modulate) ---------
        def build_hT(dst, eff_s, eff_h, mv, rstd, idx0, tg, toff, pstr):
            norms = []
            for s in range(4):
                t = toff + s
                nrm = work.tile([P, D], BF16, tag="nrm", bufs=4)
                i = idx0 + s
                nc.vector.tensor_scalar(out=nrm[:], in0=x_sb[:, t, :],
                                        scalar1=mv[:, i, 0:1],
                                        scalar2=rstd[:, i:i + 1],
                                        op0=OP.subtract, op1=OP.mult)
                norms.append(nrm)
            for k in range(KT):
                pt = pstr.tile([P, 4 * P], BF16, tag="tr")
                for s in range(4):
                    nc.tensor.transpose(pt[:, s * P:(s + 1) * P],
                                        norms[s][:, k * P:(k + 1) * P],
                                        ident[:])
                nc.vector.tensor_scalar(
                    out=dst[:, k, tg * 512:(tg + 1) * 512], in0=pt[:],
                    scalar1=eff_s[:, k:k + 1], scalar2=eff_h[:, k:k + 1],
                    op0=OP.mult, op1=OP.add)

        # ---------------- prologue: h1T + QKV (fp8 DoubleRow) -----------
        pro_d = {}
        wv_half = []

        def open_prologue():
            pro_d["pro_cm"] = tc.tile_pool(name="pro", bufs=1)
            pro = pro_d["pro_cm"].__enter__()
            h1T = pro.tile([P, KT, L], FP8, name="h1T")
            with tc.tile_pool(name="ps_tr1", bufs=1, space="PSUM") as pstr1:
                for tg in range(2):
                    build_hT(h1T, eff1s, eff1h, mv1, rstd1, tg * 4, tg,
                             tg * 4, pstr1)
            pro_d["qs_cm"] = tc.tile_pool(name="qkv_stream", bufs=3)
            pro_d["qst"] = pro_d["qs_cm"].__enter__()
            pro_d["h1T"] = h1T

        def qk_pair(qk, jp):
            fbase = D if qk == 0 else 0
            dst = k8 if qk == 0 else q8
            wj = pro_d["qst"].tile([P, KT, 2 * P], FP8, tag="wj", bufs=2,
                                   name="wjt")
            nc.sync.dma_start(
                wj[:], qkvw_d[:, :, fbase + jp * P:fbase + (jp + 2) * P])
            for jo in range(2):
                jj = jp + jo
                ci = (fbase + jj * P) // P
                for tg in range(2):
                    pq = mm_tile(jj * 2 + tg, "pq")
                    for k in range(0, KT, 2):
                        nc.tensor.matmul(
                            pq[:], wj[:, k:k + 2, jo * P:(jo + 1) * P],
                            pro_d["h1T"][:, k:k + 2,
                                         tg * 512:(tg + 1) * 512],
                            start=(k == 0), stop=(k == KT - 2),
                            perf_mode=DR)
                    nc.vector.tensor_scalar(
                        out=dst[:, jj, tg * 512:(tg + 1) * 512], in0=pq[:],
                        scalar1=dqqk[:, ci:ci + 1], scalar2=bqk[:, ci:ci + 1],
                        op0=OP.mult, op1=OP.add)

        def v_chunk(tt, fh):
            pv = mm_tile(tt * 2 + fh, "pv")
            nc.tensor.matmul(pv[:], onesrow[:, 0:P],
                             vbs[:, fh * 512:(fh + 1) * 512],
                             start=True, stop=False)
            for k in range(0, KT, 2):
                nc.tensor.matmul(pv[:],
                                 pro_d["h1T"][:, k:k + 2, tt * P:(tt + 1) * P],
                                 wv_half[fh][:, k:k + 2, :],
                                 start=False, stop=(k == KT - 2),
                                 skip_group_check=True, perf_mode=DR)
            nc.vector.tensor_tensor(
                out=v_sb[:, tt, fh * 8:(fh + 1) * 8, 0:HD], in0=pv[:],
                in1=dqvb[:, fh * 512:(fh + 1) * 512], op=OP.mult)

        def load_wv(fh):
            wv = pro_d["qst"].tile([P, KT, 512], FP8, tag="wv", bufs=1,
                          name=f"wv{fh}")
            nc.sync.dma_start(wv[:],
                              qkvw_d[:, :, 2 * D + fh * 512:
                                     2 * D + (fh + 1) * 512])
            wv_half.append(wv)

        # ---------------- attention + MLP, phase-pipelined --------------
        fsd = {}

        def fs_tile(*a, **k):
            return fsd["fs"].tile(*a, **k)

        with tc.tile_pool(name="attn", bufs=2) as ap, \
                tc.tile_pool(name="aTp", bufs=1) as aTp, \
                tc.tile_pool(name="ps_m", bufs=1, space="PSUM") as psm:

            attn_ps = {}
            attn_ps["pss_cm"] = tc.tile_pool(name="ps_s", bufs=2,
                                             space="PSUM")
            pss = attn_ps["pss_cm"].__enter__()
            attn_ps["psav_cm"] = tc.tile_pool(name="ps_av", bufs=1,
                                              space="PSUM")
            psav = attn_ps["psav_cm"].__enter__()

            def mm_tile(i, nm):
                return psm.tile([P, 512], F32, tag=f"f2_{i % 2}", bufs=1,
                                name=nm)

            open_prologue()

            def load_v(fh):
                load_wv(fh)
                for tt in range(TT):
                    v_chunk(tt, fh)

            def scores_exp(qc, j):
                q0 = qc * QC
                attA = ap.tile([P, TT, QC], FP8, tag="attA")
                attB = ap.tile([P, TT, QC], FP8, tag="attB")
                for hh, att in ((0, attA), (1, attB)):
                    p0 = hh * HD
                    for mp in range(TT // 2):
                        ps2 = pss.tile([P, 2, QC], F32, tag="sc", name="ps2")
                        for s in range(2):
                            m = 2 * mp + s
                            nc.tensor.matmul(
                                ps2[:, s, :],
                                k8[p0:p0 + HD, j, m * P:(m + 1) * P],
                                q8[p0:p0 + HD, j, q0:q0 + QC],
                                start=True, stop=True,
                                tile_position=(p0, 0))
                        nc.scalar.activation(att[:, 2 * mp:2 * mp + 2, :],
                                             ps2[:], AF.Exp, scale=ESC)
                return attA, attB

            def av_norm(j, attA, attB, aT):
                for hh, att in ((0, attA), (1, attB)):
                    h = 2 * j + hh
                    pu = psav.tile([HD + 1, QC], F32, tag="pu")
                    for mp in range(TT // 2):
                        nc.tensor.matmul(
                            pu[:], v_sb[:, 2 * mp:2 * mp + 2, h, :],
                            att[:, 2 * mp:2 * mp + 2, :],
                            start=(mp == 0), stop=(mp == TT // 2 - 1),
                            perf_mode=DR)
                    drow = work.tile([1, QC], F32, tag="drow", bufs=1)
                    nc.vector.tensor_copy(drow[:], pu[HD:HD + 1, :])
                    dbc = work.tile([HD, QC], F32, tag="dbc", bufs=1)
                    nc.gpsimd.partition_broadcast(dbc[:], drow[:])
                    rec = work.tile([HD, QC], F32, tag="rec", bufs=1)
                    nc.vector.reciprocal_approx_fast(rec[:], dbc[:])
                    nc.vector.tensor_tensor(
                        out=aT[hh * 64:(hh + 1) * 64, j, :],
                        in0=pu[0:HD, :], in1=rec[:], op=OP.mult)

            def proj_block(qc, aT, s):
                t_global = qc * 4 + s
                for fh in range(2):
                    pp = mm_tile(s * 2 + fh, "pp")
                    nc.tensor.matmul(pp[:], onesrow[:, 0:P],
                                     pbs[:, fh * 512:(fh + 1) * 512],
                                     start=True, stop=False)
                    for k in range(KT):
                        nc.tensor.matmul(
                            pp[:], aT[:, k, s * P:(s + 1) * P],
                            projw[:, k, fh * 512:(fh + 1) * 512],
                            start=False, stop=(k == KT - 1),
                            skip_group_check=True)
                    tmp = work.tile([P, 512], BF16, tag="ptmp", bufs=1)
                    nc.vector.tensor_tensor(
                        out=tmp[:], in0=pp[:],
                        in1=g1bc[:, fh * 512:(fh + 1) * 512], op=OP.mult)
                    nc.gpsimd.tensor_add(
                        x_sb[:, t_global, fh * 512:(fh + 1) * 512],
                        x_sb[:, t_global, fh * 512:(fh + 1) * 512],
                        tmp[:])

            def fc1_gelu(qc):
                # 4-bank pf rotation: PE stays 4 gelu-evictions ahead so
                # the scheduler never slots attn exps between gelus (which
                # would thrash the ACT table)
                for fcp in range(0, FT, 4):
                    w1 = fs_tile([P, KT, 4 * P], BF16, tag="w1", bufs=3,
                                 name="w1t")
                    nc.sync.dma_start(w1[:],
                                      fc1w_d[:, :, fcp * P:(fcp + 4) * P])
                    if qc == 0:
                        sc = pss.tile([P, 2, QC], F32, tag="sc",
                                      name="pfsc")
                        extra = [sc[:, 0, :], sc[:, 1, :]]
                    else:
                        tl0 = fsd["tail"].tile([P, 512], F32, tag="tl0",
                                               bufs=1, name="pft0")
                        tl1 = fsd["tail"].tile([P, 512], F32, tag="tl1",
                                               bufs=1, name="pft1")
                        extra = [tl0[:], tl1[:]]
                    pfs = [mm_tile(0, "pf")[:], mm_tile(1, "pf")[:],
                           extra[0], extra[1]]
                    for i in range(4):
                        fc = fcp + i
                        pf = pfs[i]
                        for k in range(KT):
                            nc.tensor.matmul(pf,
                                             w1[:, k, i * P:(i + 1) * P],
                                             h2T[:, k, :],
                                             start=(k == 0),
                                             stop=(k == KT - 1))
                        nc.scalar.activation(geluT[:, fc, :], pf,
                                             AF.Gelu_apprx_tanh,
                                             bias=bf1T[:, fc:fc + 1])

            def fc2_blocks(qc):
                # s-pairs share the streamed w2 chunk; 2 psum banks live
                for sp in range(2):
                    for fp in range(2):
                        def blk(sp=sp, fp=fp, qc=qc):
                            ps2 = [mm_tile(0, "pf2a"), mm_tile(1, "pf2b")]
                            for s in range(2):
                                nc.tensor.matmul(
                                    ps2[s][:], onesrow[:, 0:P],
                                    f2bs[:, fp * 512:(fp + 1) * 512],
                                    start=True, stop=False)
                            for fpr in range(FT // 2):
                                w2 = fs_tile([P, 2, 512], BF16, tag="w2",
                                             bufs=3, name="w2t")
                                nc.sync.dma_start(
                                    w2[:], fc2w_d[:, 2 * fpr:2 * fpr + 2,
                                                  fp * 512:(fp + 1) * 512])
                                for i in range(2):
                                    ft = 2 * fpr + i
                                    for s in range(2):
                                        tok = sp * 2 + s
                                        nc.tensor.matmul(
                                            ps2[s][:],
                                            geluT[:, ft,
                                                  tok * P:(tok + 1) * P],
                                            w2[:, i, :],
                                            start=False,
                                            stop=(ft == FT - 1),
                                            skip_group_check=True)
                            for s in range(2):
                                t_global = qc * 4 + sp * 2 + s
                                tmp = work.tile([P, 512], BF16, tag="ftmp",
                                                bufs=1)
                                nc.vector.tensor_tensor(
                                    out=tmp[:], in0=ps2[s][:],
                                    in1=g2bc[:, fp * 512:(fp + 1) * 512],
                                    op=OP.mult)
                                oth = work.tile([P, 512], F32, tag="ot",
                                                bufs=2,
                                                name=f"ot{qc}{fp}{sp}{s}")
                                nc.gpsimd.tensor_add(
                                    oth[:],
                                    x_sb[:, t_global,
                                         fp * 512:(fp + 1) * 512],
                                    tmp[:])
                                nc.sync.dma_start(
                                    out_v[:, t_global,
                                          fp * 512:(fp + 1) * 512],
                                    oth[:])
                        yield blk

            def fc2_tail_blocks(qc):
                # post-attn: stream each w2 chunk ONCE for all 4 token
                # blocks (4 live psum banks) -- halves fc2 HBM traffic
                for fp in range(2):
                    def blk(fp=fp, qc=qc):
                        tl = fsd["tail"]
                        ps4 = [tl.tile([P, 512], F32, tag=f"tl{i}", bufs=1,
                                       name=f"ptl{i}") for i in range(4)]
                        for s in range(4):
                            nc.tensor.matmul(
                                ps4[s][:], onesrow[:, 0:P],
                                f2bs[:, fp * 512:(fp + 1) * 512],
                                start=True, stop=False)
                        for fpr in range(FT // 2):
                            w2 = fs_tile([P, 2, 512], BF16, tag="w2",
                                         bufs=3, name="w2t")
                            nc.sync.dma_start(
                                w2[:], fc2w_d[:, 2 * fpr:2 * fpr + 2,
                                              fp * 512:(fp + 1) * 512])
                            for i in range(2):
                                ft = 2 * fpr + i
                                for s in range(4):
                                    nc.tensor.matmul(
                                        ps4[s][:],
                                        geluT[:, ft, s * P:(s + 1) * P],
                                        w2[:, i, :],
                                        start=False, stop=(ft == FT - 1),
                                        skip_group_check=True)
                        for s in range(4):
                            t_global = qc * 4 + s
                            tmp = work.tile([P, 512], BF16, tag="ftmp",
                                            bufs=1)
                            nc.vector.tensor_tensor(
                                out=tmp[:], in0=ps4[s][:],
                                in1=g2bc[:, fp * 512:(fp + 1) * 512],
                                op=OP.mult)
                            oth = work.tile([P, 512], F32, tag="ot",
                                            bufs=2, name=f"otl{qc}{fp}{s}")
                            nc.gpsimd.tensor_add(
                                oth[:],
                                x_sb[:, t_global, fp * 512:(fp + 1) * 512],
                                tmp[:])
                            nc.sync.dma_start(
                                out_v[:, t_global, fp * 512:(fp + 1) * 512],
                                oth[:])
                    yield blk

            closed = {}

            def run_qc(qc, pending, producers=None):
                pend_i = 0
                with nc.named_scope(f"attn{qc}"):
                    aT = aTp.tile([P, KT, QC], BF16, tag="aT",
                                  name=f"aT_{qc}")
                    atts = []
                    for j in range(KT):
                        if producers:
                            for fn in producers.pop(j, []):
                                fn()
                        atts.append(scores_exp(qc, j))
                        if j % 2 == 0 and pend_i < len(pending):
                            pending[pend_i]()
                            pend_i += 1
                        if j >= 1:
                            av_norm(j - 1, *atts[j - 1], aT)
                    av_norm(KT - 1, *atts[KT - 1], aT)
                if qc == 0 and not closed:
                    # h1T / V weights are dead: free prologue SBUF
                    pro_d["qs_cm"].__exit__(None, None, None)
                    pro_d["pro_cm"].__exit__(None, None, None)
                    fsd["cm"] = tc.tile_pool(name="fc_stream", bufs=2)
                    fsd["fs"] = fsd["cm"].__enter__()
                    closed["done"] = True
                if qc == 1:
                    # scores/av psum dead: free 5 banks for the 4-bank tail
                    attn_ps["psav_cm"].__exit__(None, None, None)
                    attn_ps["pss_cm"].__exit__(None, None, None)
                    fsd["tail_cm"] = tc.tile_pool(name="ps_tail", bufs=1,
                                                  space="PSUM")
                    fsd["tail"] = fsd["tail_cm"].__enter__()
                with nc.named_scope(f"proj{qc}"):
                    mv2 = work.tile([P, 4, 2], F32, tag="mv4", name="mv4")
                    for s in range(4):
                        proj_block(qc, aT, s)
                        # ln2 stats for this tile right away (fills the
                        # proj->ln2 dependency valley)
                        stats = work.tile([P, 2, 6], F32, tag="stats")
                        x_ap = x_sb[:, qc * 4 + s, :]
                        for sg in range(2):
                            nc.vector.bn_stats(
                                stats[:, sg, :],
                                x_ap[:, sg * 512:(sg + 1) * 512])
                        nc.vector.bn_aggr(mv2[:, s, :], stats[:])
                with nc.named_scope(f"ln2_{qc}"):
                    rstd2 = work.tile([P, 4], F32, tag="rstd4",
                                      name="rstd4")
                    nc.scalar.activation(rstd2[:], mv2[:, :, 1], AF.Sqrt,
                                         bias=eps_sb[:])
                    nc.vector.reciprocal(rstd2[:], rstd2[:])
                    with tc.tile_pool(name="ps_tr2", bufs=1,
                                      space="PSUM") as pstr2:
                        build_hT(h2T, eff2s, eff2h, mv2, rstd2, 0, 0,
                                 qc * 4, pstr2)
                with nc.named_scope(f"fc1_{qc}"):
                    fc1_gelu(qc)
                    while pend_i < len(pending):
                        pending[pend_i]()
                        pend_i += 1
                if qc == 1:
                    return list(fc2_tail_blocks(qc))
                return list(fc2_blocks(qc))

            producers0 = {
                0: [lambda: qk_pair(0, 0), lambda: qk_pair(1, 0),
                    lambda: load_v(0)],
                1: [lambda: qk_pair(0, 2), lambda: qk_pair(1, 2)],
                2: [lambda: qk_pair(0, 4), lambda: qk_pair(1, 4),
                    lambda: load_v(1)],
                3: [lambda: qk_pair(0, 6), lambda: qk_pair(1, 6)],
            }
            pending = run_qc(0, [], producers0)
            pending = run_qc(1, pending)
            with nc.named_scope("mlp_tail"):
                for blk in pending:
                    blk()
            fsd["tail_cm"].__exit__(None, None, None)
            fsd["cm"].__exit__(None, None, None)

        _work_cm.__exit__(None, None, None)

    nc.compile()
    return nc, names


def _get_compiled():
    if "nc" not in _CACHE:
        _CACHE["nc"], _CACHE["names"] = _build()
    return _CACHE["nc"], _CACHE["names"]


def _q8col(w, smax=224.0):
    w = np.asarray(w, np.float32)
    am = np.abs(w).max(axis=0, keepdims=True)
    s = np.where(am > 0, smax / np.maximum(am, 1e-30), 1.0)
    w8 = np.clip(w * s, -240, 240).astype(F8NP)
    return w8, s[0]


def _pmajor(w):
    w = np.asarray(w)
    kp, n = w.shape
    return np.ascontiguousarray(w.reshape(kp // P, P, n).transpose(1, 0, 2))


def _prep_maps(names, x, c, ln1_w, ln1_b, ln2_w, ln2_b, ada_w, ada_b,
               qkv_w, qkv_b, proj_w, proj_b, fc1_w, fc1_b, fc2_w, fc2_b):
    x = np.asarray(x, np.float32)
    c = np.asarray(c, np.float32)
    qkv8, s_qkv = _q8col(qkv_w)
    proj8, s_proj = _q8col(proj_w)
    qkv_b = np.asarray(qkv_b, np.float32)
    s_wv = s_qkv[2 * D:]

    def tcols(v):
        return np.asarray(v, np.float32).reshape(KT, P).T
    lnT = np.concatenate([tcols(ln1_w), tcols(ln1_b),
                          tcols(ln2_w), tcols(ln2_b)], axis=1)

    common = {
        names["lnT"]: np.ascontiguousarray(lnT, np.float32),
        names["ada_w"]: _pmajor(ada_w).astype(BF),
        names["ada_b"]: np.asarray(ada_b).astype(BF).reshape(1, -1),
        names["qkv_w"]: _pmajor(qkv8),
        names["bqk"]: np.ascontiguousarray(
            (qkv_b[0:2 * D] * S_QK).reshape(16, P).T.astype(np.float32)),
        names["dqqk"]: np.ascontiguousarray(
            (S_QK / s_qkv[0:2 * D]).reshape(16, P).T.astype(np.float32)),
        names["vbs"]: (qkv_b[2 * D:] * s_wv).astype(BF).reshape(1, D),
        names["dqv"]: (S_V / s_wv).astype(BF).reshape(1, D),
        names["proj_w"]: _pmajor(proj8),
        names["pbs"]: (np.asarray(proj_b, np.float32) * S_V * s_proj)
        .astype(BF).reshape(1, D),
        names["dqp"]: (1.0 / (S_V * s_proj)).astype(BF).reshape(1, D),
        names["fc1_w"]: _pmajor(fc1_w).astype(BF),
        names["bf1T"]: np.ascontiguousarray(
            np.asarray(fc1_b, np.float32).reshape(FT, P).T),
        names["fc2_w"]: _pmajor(fc2_w).astype(BF),
        names["f2bs"]: np.asarray(fc2_b).astype(BF).reshape(1, D),
    }
    in_maps = []
    for b in range(B):
        m = dict(common)
        m[names["x"]] = np.ascontiguousarray(
            x[b].reshape(TT, P, D).transpose(1, 0, 2)).astype(BF)
        m[names["c"]] = np.ascontiguousarray(c[b].reshape(KT, P).T)
        in_maps.append(m)
    return in_maps


def kernel(x, c, ln1_w, ln1_b, ln2_w, ln2_b, ada_w, ada_b,
           qkv_w, qkv_b, proj_w, proj_b, fc1_w, fc1_b, fc2_w, fc2_b,
           _trace=False):
    nc, names = _get_compiled()
    in_maps = _prep_maps(names, x, c, ln1_w, ln1_b, ln2_w, ln2_b,
                         ada_w, ada_b, qkv_w, qkv_b, proj_w, proj_b,
                         fc1_w, fc1_b, fc2_w, fc2_b)
    res = bass_utils.run_bass_kernel_spmd(nc, in_maps, core_ids=list(range(B)),
                                          trace=_trace)
    out = np.stack([res.results[b][names["out"]] for b in range(B)])
    if _trace:
        _CACHE["last_result"] = res
    return out


# revision 42
# speedup vs baseline: 1.2028x; 1.1896x over previous
"""DiT block kernel for 8 Trainium2 NeuronCores.

Sharding: data-parallel over batch (B=8 -> one batch element per core).

Vs the 856us baseline:
  - QKV + attn@V run fp8e4 DoubleRow (2 k-tiles per pass, 0.5 cyc/row):
    h1T cast fp8 (scale 1), qkv_w fp8 per-column, v_sb fp8 (xS_V),
    attn probs fp8 (as before). MLP + ada stay bf16 (fp8 there blows
    the 2e-2 error budget; measured by numpy ablation).
  - exp batched over two PSUM banks ([128,1024] per ACT instruction)
    to amortize the ~175ns fixed ACT overhead; halves instruction count.
  - fc1 bias moved into the gelu activation (per-partition bias AP);
    kills 64 ones-row bias matmuls.
  - small row copies / residual adds moved to the idle GPSIMD engine
    to unload DVE (drow eviction, v eviction, proj/fc2 residual adds).
  - x kept bf16 in SBUF (halves LN/norm DVE traffic and x DMA).
"""

import sys

sys.path.insert(0, "/opt/trn_rl_repo")

import numpy as np
import ml_dtypes

import concourse.bacc as bacc
import concourse.tile as tile
from concourse import mybir
from concourse import bass_utils
from concourse.masks import make_identity

F32 = mybir.dt.float32
BF16 = mybir.dt.bfloat16
FP8 = mybir.dt.float8e4
BF = ml_dtypes.bfloat16
F8NP = ml_dtypes.float8_e4m3
OP = mybir.AluOpType
AF = mybir.ActivationFunctionType
DR = mybir.MatmulPerfMode.DoubleRow

B = 8
L = 1024
D = 1024
H = 16
HD = 64
FF = 4096
LN_EPS = 1e-5
P = 128
TT = L // P
KT = D // P
FT = FF // P
QC = 512
NQC = L // QC

S_QK = 32.0          # q/k fp8 scale
S_V = 32.0           # v fp8 scale
ESC = 0.125 / (S_QK * S_QK)

_CACHE = {}


def _build():
    nc = bacc.Bacc(None, target_bir_lowering=False)
    names = {}
    with tile.TileContext(nc) as tc, \
            tc.tile_pool(name="dram", bufs=1, space="DRAM") as dram, \
            tc.tile_pool(name="per", bufs=1) as per:

        x_d = dram.tile([P, TT, D], BF16, kind="ExternalInput", name="x")
        c_d = dram.tile([P, KT], F32, kind="ExternalInput", name="c")
        lnT_d = dram.tile([P, 4 * KT], F32, kind="ExternalInput", name="lnT")
        adaw_d = dram.tile([P, KT, 6 * D], BF16, kind="ExternalInput",
                           name="ada_w")
        adab_d = dram.tile([1, 6 * D], BF16, kind="ExternalInput",
                           name="ada_b")
        qkvw_d = dram.tile([P, KT, 3 * D], FP8, kind="ExternalInput",
                           name="qkv_w")
        bqk_d = dram.tile([P, 16], F32, kind="ExternalInput", name="bqk")
        dqqk_d = dram.tile([P, 16], F32, kind="ExternalInput", name="dqqk")
        vbs_d = dram.tile([1, D], BF16, kind="ExternalInput", name="vbs")
        dqv_d = dram.tile([1, D], BF16, kind="ExternalInput", name="dqv")
        projw_d = dram.tile([P, KT, D], FP8, kind="ExternalInput",
                            name="proj_w")
        pbs_d = dram.tile([1, D], BF16, kind="ExternalInput", name="pbs")
        dqp_d = dram.tile([1, D], BF16, kind="ExternalInput", name="dqp")
        fc1w_d = dram.tile([P, KT, FF], BF16, kind="ExternalInput",
                           name="fc1_w")
        bf1T_d = dram.tile([P, FT], F32, kind="ExternalInput", name="bf1T")
        fc2w_d = dram.tile([P, FT, D], BF16, kind="ExternalInput",
                           name="fc2_w")
        f2bs_d = dram.tile([1, D], BF16, kind="ExternalInput", name="f2bs")
        out_d = dram.tile([L, D], F32, kind="ExternalOutput", name="out")
        for t, n in [(x_d, "x"), (c_d, "c"), (lnT_d, "lnT"),
                     (adaw_d, "ada_w"), (adab_d, "ada_b"),
                     (qkvw_d, "qkv_w"), (bqk_d, "bqk"), (dqqk_d, "dqqk"),
                     (vbs_d, "vbs"), (dqv_d, "dqv"),
                     (projw_d, "proj_w"), (pbs_d, "pbs"), (dqp_d, "dqp"),
                     (fc1w_d, "fc1_w"), (bf1T_d, "bf1T"),
                     (fc2w_d, "fc2_w"), (f2bs_d, "f2bs"), (out_d, "out")]:
            names[n] = t.name
        out_v = out_d[:].rearrange("(t p) d -> p t d", p=P)

        x_sb = per.tile([P, TT, D], BF16)
        for t in range(TT):
            eng = nc.sync if t < 4 else nc.scalar
            eng.dma_start(x_sb[:, t, :], x_d[:, t, :])
        ident = per.tile([P, P], BF16)
        make_identity(nc, ident[:])
        onesrow = per.tile([1, 512], BF16)
        nc.vector.memset(onesrow[:], 1.0)
        eps_sb = per.tile([P, 1], F32)
        nc.vector.memset(eps_sb[:], LN_EPS)

        lnT = per.tile([P, 4 * KT], F32)
        nc.sync.dma_start(lnT[:], lnT_d[:])
        bqk = per.tile([P, 16], F32)
        nc.sync.dma_start(bqk[:], bqk_d[:])
        dqqk = per.tile([P, 16], F32)
        nc.sync.dma_start(dqqk[:], dqqk_d[:])
        vbs = per.tile([1, D], BF16)
        nc.sync.dma_start(vbs[:], vbs_d[:])
        dqv_row = per.tile([1, D], BF16)
        nc.sync.dma_start(dqv_row[:], dqv_d[:])
        pbs = per.tile([1, D], BF16)
        nc.sync.dma_start(pbs[:], pbs_d[:])
        dqp_row = per.tile([1, D], BF16)
        nc.sync.dma_start(dqp_row[:], dqp_d[:])

        f2bs = per.tile([1, D], BF16)
        nc.sync.dma_start(f2bs[:], f2bs_d[:])
        bf1T = per.tile([P, FT], F32)
        nc.sync.dma_start(bf1T[:], bf1T_d[:])
        adab = per.tile([1, 6 * D], BF16)
        nc.sync.dma_start(adab[:], adab_d[:])
        projw = per.tile([P, KT, D], FP8)
        for k in range(KT):
            # ACT-engine HW DGE queue: keeps the sync queue free for the
            # startup-critical x / ada / qkv weight streams
            nc.scalar.dma_start(projw[:, k, :], projw_d[:, k, :])
        dqvb = per.tile([P, D], BF16)
        nc.gpsimd.partition_broadcast(dqvb[:], dqv_row[:])

        eff1s = per.tile([P, KT], F32)
        eff1h = per.tile([P, KT], F32)
        eff2s = per.tile([P, KT], F32)
        eff2h = per.tile([P, KT], F32)
        g1bc = per.tile([P, D], BF16)
        g2bc = per.tile([P, D], BF16)

        q8 = per.tile([P, KT, L], FP8)
        k8 = per.tile([P, KT, L], FP8)
        v_sb = per.tile([P, TT, H, HD + 1], FP8)
        nc.vector.memset(v_sb[:, :, :, HD:HD + 1], 1.0)
        h2T = per.tile([P, KT, QC], BF16)
        geluT = per.tile([P, FT, QC], BF16)

        _work_cm = tc.tile_pool(name="work", bufs=2)
        work = _work_cm.__enter__()

        def ln_stats(x_aps):
            n = len(x_aps)
            mv = work.tile([P, n, 2], F32, tag=f"mv{n}", name=f"mv{n}")
            for i, x_ap in enumerate(x_aps):
                stats = work.tile([P, 2, 6], F32, tag="stats")
                for sg in range(2):
                    nc.vector.bn_stats(stats[:, sg, :],
                                       x_ap[:, sg * 512:(sg + 1) * 512])
                nc.vector.bn_aggr(mv[:, i, :], stats[:])
            rstd = work.tile([P, n], F32, tag=f"rstd{n}", name=f"rstd{n}")
            nc.scalar.activation(rstd[:], mv[:, :, 1], AF.Sqrt, bias=eps_sb[:])
            nc.vector.reciprocal(rstd[:], rstd[:])
            return mv, rstd

        mv1, rstd1 = ln_stats([x_sb[:, t, :] for t in range(TT)])

        # ---------------- ada (bf16, 512-wide moving weights) -----------
        with nc.named_scope("ada"), \
                tc.tile_pool(name="ada_sb", bufs=1) as asb, \
                tc.tile_pool(name="ada_stream", bufs=2) as ast, \
                tc.tile_pool(name="ps_ada", bufs=2, space="PSUM") as psa, \
                tc.tile_pool(name="ps_t6", bufs=1, space="PSUM") as pst6:
            c_sb = asb.tile([P, KT], F32)
            nc.sync.dma_start(c_sb[:], c_d[:])
            silu_b = asb.tile([P, KT], BF16)
            nc.scalar.activation(silu_b[:], c_sb[:], AF.Silu)
            ssgb = asb.tile([1, 6 * D], BF16, name="ssgb")

            def ada_chunk(nch, eng=None):
                aw = ast.tile([P, KT, 512], BF16, tag="aw")
                (eng or nc.sync).dma_start(
                    aw[:], adaw_d[:, :, nch * 512:(nch + 1) * 512])
                pa = psa.tile([1, 512], F32, tag="ada")
                nc.tensor.matmul(pa[:], onesrow[:, 0:1],
                                 adab[:, nch * 512:(nch + 1) * 512],
                                 start=True, stop=False)
                for k in range(KT):
                    nc.tensor.matmul(pa[:], silu_b[:, k:k + 1], aw[:, k, :],
                                     start=False, stop=(k == KT - 1),
                                     skip_group_check=True)
                nc.vector.tensor_copy(
                    ssgb[:, nch * 512:(nch + 1) * 512], pa[:])

            # shift/scale rows -> feature-major cols; v' = sh1,sc1,sh2,sc2
            pt6 = pst6.tile([P, 32, 2], BF16)

            def ada_tr(vi, v):
                for ch in range(KT):
                    i = ch * 4 + vi
                    nc.tensor.transpose(
                        pt6[:, i, 0:1],
                        ssgb[0:1, v * D + ch * P:v * D + (ch + 1) * P],
                        ident[0:1, 0:1])
                nc.vector.tensor_copy(ssgT[:, :, vi], pt6[:, vi:32:4, 0])

            ssgT = asb.tile([P, KT, 4], F32, name="ssgT")
            t0 = asb.tile([P, KT], F32, name="t0")
            t1 = asb.tile([P, KT], F32, name="t1")
            # chunk order puts the attn shift/scale (-> h1T -> qkv) first
            for nch in (0, 1, 2, 3):
                ada_chunk(nch)
            ada_tr(0, 0)
            ada_tr(1, 1)
            nc.vector.tensor_scalar_add(t0[:], ssgT[:, :, 1], 1.0)
            nc.vector.tensor_mul(eff1s[:], t0[:], lnT[:, 0:KT])
            nc.vector.tensor_mul(t1[:], t0[:], lnT[:, KT:2 * KT])
            nc.vector.tensor_add(eff1h[:], t1[:], ssgT[:, :, 0])
            for nch in (6, 7, 8, 9):
                ada_chunk(nch)
            ada_tr(2, 3)
            ada_tr(3, 4)
            nc.vector.tensor_scalar_add(t0[:], ssgT[:, :, 3], 1.0)
            nc.vector.tensor_mul(eff2s[:], t0[:], lnT[:, 2 * KT:3 * KT])
            nc.vector.tensor_mul(t1[:], t0[:], lnT[:, 3 * KT:4 * KT])
            nc.vector.tensor_add(eff2h[:], t1[:], ssgT[:, :, 2])
            for nch in (4, 5, 10, 11):
                ada_chunk(nch)
            g1row = asb.tile([1, D], BF16, name="g1row")
            nc.vector.tensor_mul(g1row[:], ssgb[:, 2 * D:3 * D], dqp_row[:])
            nc.gpsimd.partition_broadcast(g1bc[:], g1row[:])
            nc.gpsimd.partition_broadcast(g2bc[:], ssgb[:, 5 * D:6 * D])

        # ---------------- LN -> hT (transpose + fused modulate) ---------
        def build_hT(dst, eff_s, eff_h, mv, rstd, idx0, tg, toff, pstr):
            norms = []
            for s in range(4):
                t = toff + s
                nrm = work.tile([P, D], BF16, tag="nrm", bufs=4)
                i = idx0 + s
                nc.vector.tensor_scalar(out=nrm[:], in0=x_sb[:, t, :],
                                        scalar1=mv[:, i, 0:1],
                                        scalar2=rstd[:, i:i + 1],
                                        op0=OP.subtract, op1=OP.mult)
                norms.append(nrm)
            for k in range(KT):
                pt = pstr.tile([P, 4 * P], BF16, tag="tr")
                for s in range(4):
                    nc.tensor.transpose(pt[:, s * P:(s + 1) * P],
                                        norms[s][:, k * P:(k + 1) * P],
                                        ident[:])
                nc.vector.tensor_scalar(
                    out=dst[:, k, tg * 512:(tg + 1) * 512], in0=pt[:],
                    scalar1=eff_s[:, k:k + 1], scalar2=eff_h[:, k:k + 1],
                    op0=OP.mult, op1=OP.add)

        # ---------------- prologue: h1T + QKV (fp8 DoubleRow) -----------
        pro_d = {}
        wv_half = []

        def open_prologue():
            pro_d["pro_cm"] = tc.tile_pool(name="pro", bufs=1)
            pro = pro_d["pro_cm"].__enter__()
            h1T = pro.tile([P, KT, L], FP8, name="h1T")
            with tc.tile_pool(name="ps_tr1", bufs=1, space="PSUM") as pstr1:
                for tg in range(2):
                    build_hT(h1T, eff1s, eff1h, mv1, rstd1, tg * 4, tg,
                             tg * 4, pstr1)
            pro_d["qs_cm"] = tc.tile_pool(name="qkv_stream", bufs=3)
            pro_d["qst"] = pro_d["qs_cm"].__enter__()
            pro_d["h1T"] = h1T

        def qk_pair(qk, jp):
            fbase = D if qk == 0 else 0
            dst = k8 if qk == 0 else q8
            wj = pro_d["qst"].tile([P, KT, 2 * P], FP8, tag="wj", bufs=2,
                                   name="wjt")
            nc.sync.dma_start(
                wj[:], qkvw_d[:, :, fbase + jp * P:fbase + (jp + 2) * P])
            for jo in range(2):
                jj = jp + jo
                ci = (fbase + jj * P) // P
                for tg in range(2):
                    pq = mm_tile(jj * 2 + tg, "pq")
                    for k in range(0, KT, 2):
                        nc.tensor.matmul(
                            pq[:], wj[:, k:k + 2, jo * P:(jo + 1) * P],
                            pro_d["h1T"][:, k:k + 2,
                                         tg * 512:(tg + 1) * 512],
                            start=(k == 0), stop=(k == KT - 2),
                            perf_mode=DR)
                    nc.vector.tensor_scalar(
                        out=dst[:, jj, tg * 512:(tg + 1) * 512], in0=pq[:],
                        scalar1=dqqk[:, ci:ci + 1], scalar2=bqk[:, ci:ci + 1],
                        op0=OP.mult, op1=OP.add)

        def v_chunk(tt, fh):
            pv = mm_tile(tt * 2 + fh, "pv")
            nc.tensor.matmul(pv[:], onesrow[:, 0:P],
                             vbs[:, fh * 512:(fh + 1) * 512],
                             start=True, stop=False)
            for k in range(0, KT, 2):
                nc.tensor.matmul(pv[:],
                                 pro_d["h1T"][:, k:k + 2, tt * P:(tt + 1) * P],
                                 wv_half[fh][:, k:k + 2, :],
                                 start=False, stop=(k == KT - 2),
                                 skip_group_check=True, perf_mode=DR)
            nc.vector.tensor_tensor(
                out=v_sb[:, tt, fh * 8:(fh + 1) * 8, 0:HD], in0=pv[:],
                in1=dqvb[:, fh * 512:(fh + 1) * 512], op=OP.mult)

        def load_wv(fh):
            wv = pro_d["qst"].tile([P, KT, 512], FP8, tag="wv", bufs=1,
                          name=f"wv{fh}")
            nc.sync.dma_start(wv[:],
                              qkvw_d[:, :, 2 * D + fh * 512:
                                     2 * D + (fh + 1) * 512])
            wv_half.append(wv)

        # ---------------- attention + MLP, phase-pipelined --------------
        fsd = {}

        def fs_tile(*a, **k):
            return fsd["fs"].tile(*a, **k)

        with tc.tile_pool(name="attn", bufs=2) as ap, \
                tc.tile_pool(name="aTp", bufs=1) as aTp, \
                tc.tile_pool(name="ps_m", bufs=1, space="PSUM") as psm:

            attn_ps = {}
            attn_ps["pss_cm"] = tc.tile_pool(name="ps_s", bufs=2,
                                             space="PSUM")
            pss = attn_ps["pss_cm"].__enter__()
            attn_ps["psav_cm"] = tc.tile_pool(name="ps_av", bufs=1,
                                              space="PSUM")
            psav = attn_ps["psav_cm"].__enter__()

            def mm_tile(i, nm):
                return psm.tile([P, 512], F32, tag=f"f2_{i % 2}", bufs=1,
                                name=nm)

            open_prologue()

            def load_v(fh):
                load_wv(fh)
                for tt in range(TT):
                    v_chunk(tt, fh)

            def scores_exp(qc, j):
                q0 = qc * QC
                attA = ap.tile([P, TT, QC], FP8, tag="attA")
                attB = ap.tile([P, TT, QC], FP8, tag="attB")
                for hh, att in ((0, attA), (1, attB)):
                    p0 = hh * HD
                    for mp in range(TT // 2):
                        ps2 = pss.tile([P, 2, QC], F32, tag="sc", name="ps2")
                        for s in range(2):
                            m = 2 * mp + s
                            nc.tensor.matmul(
                                ps2[:, s, :],
                                k8[p0:p0 + HD, j, m * P:(m + 1) * P],
                                q8[p0:p0 + HD, j, q0:q0 + QC],
                                start=True, stop=True,
                                tile_position=(p0, 0))
                        nc.scalar.activation(att[:, 2 * mp:2 * mp + 2, :],
                                             ps2[:], AF.Exp, scale=ESC)
                return attA, attB

            def av_norm(j, attA, attB, aT):
                for hh, att in ((0, attA), (1, attB)):
                    h = 2 * j + hh
                    pu = psav.tile([HD + 1, QC], F32, tag="pu")
                    for mp in range(TT // 2):
                        nc.tensor.matmul(
                            pu[:], v_sb[:, 2 * mp:2 * mp + 2, h, :],
                            att[:, 2 * mp:2 * mp + 2, :],
                            start=(mp == 0), stop=(mp == TT // 2 - 1),
                            perf_mode=DR)
                    drow = work.tile([1, QC], F32, tag="drow", bufs=1)
                    nc.vector.tensor_copy(drow[:], pu[HD:HD + 1, :])
                    dbc = work.tile([HD, QC], F32, tag="dbc", bufs=1)
                    nc.gpsimd.partition_broadcast(dbc[:], drow[:])
                    rec = work.tile([HD, QC], F32, tag="rec", bufs=1)
                    nc.vector.reciprocal_approx_fast(rec[:], dbc[:])
                    nc.vector.tensor_tensor(
                        out=aT[hh * 64:(hh + 1) * 64, j, :],
                        in0=pu[0:HD, :], in1=rec[:], op=OP.mult)

            def proj_block(qc, aT, s):
                t_global = qc * 4 + s
                for fh in range(2):
                    pp = mm_tile(s * 2 + fh, "pp")
                    nc.tensor.matmul(pp[:], onesrow[:, 0:P],
                                     pbs[:, fh * 512:(fh + 1) * 512],
                                     start=True, stop=False)
                    for k in range(KT):
                        nc.tensor.matmul(
                            pp[:], aT[:, k, s * P:(s + 1) * P],
                            projw[:, k, fh * 512:(fh + 1) * 512],
                            start=False, stop=(k == KT - 1),
                            skip_group_check=True)
                    tmp = work.tile([P, 512], BF16, tag="ptmp", bufs=1)
                    nc.vector.tensor_tensor(
                        out=tmp[:], in0=pp[:],
                        in1=g1bc[:, fh * 512:(fh + 1) * 512], op=OP.mult)
                    nc.gpsimd.tensor_add(
                        x_sb[:, t_global, fh * 512:(fh + 1) * 512],
                        x_sb[:, t_global, fh * 512:(fh + 1) * 512],
                        tmp[:])

            def fc1_gelu(qc):
                # psum evicts through a fast DVE copy; the gelus then read
                # stable SBUF and batch AFTER all evictions -- keeps attn
                # exps from interleaving (ACT table thrash) while the fc1
                # matmuls still overlap the attention phases
                for fcp in range(0, FT, 4):
                    w1 = fs_tile([P, KT, 4 * P], BF16, tag="w1", bufs=3,
                                 name="w1t")
                    nc.sync.dma_start(w1[:],
                                      fc1w_d[:, :, fcp * P:(fcp + 4) * P])
                    for i in range(4):
                        fc = fcp + i
                        pf = mm_tile(fc, "pf")
                        for k in range(KT):
                            nc.tensor.matmul(pf[:],
                                             w1[:, k, i * P:(i + 1) * P],
                                             h2T[:, k, :],
                                             start=(k == 0),
                                             stop=(k == KT - 1))
                        nc.vector.tensor_copy(geluT[:, fc, :], pf[:])
                for fc in range(FT):
                    nc.scalar.activation(geluT[:, fc, :], geluT[:, fc, :],
                                         AF.Gelu_apprx_tanh,
                                         bias=bf1T[:, fc:fc + 1])

            def fc2_blocks(qc):
                # s-pairs share the streamed w2 chunk; 2 psum banks live
                for sp in range(2):
                    for fp in range(2):
                        def blk(sp=sp, fp=fp, qc=qc):
                            ps2 = [mm_tile(0, "pf2a"), mm_tile(1, "pf2b")]
                            for s in range(2):
                                nc.tensor.matmul(
                                    ps2[s][:], onesrow[:, 0:P],
                                    f2bs[:, fp * 512:(fp + 1) * 512],
                                    start=True, stop=False)
                            for fpr in range(FT // 2):
                                w2 = fs_tile([P, 2, 512], BF16, tag="w2",
                                             bufs=3, name="w2t")
                                nc.sync.dma_start(
                                    w2[:], fc2w_d[:, 2 * fpr:2 * fpr + 2,
                                                  fp * 512:(fp + 1) * 512])
                                for i in range(2):
                                    ft = 2 * fpr + i
                                    for s in range(2):
                                        tok = sp * 2 + s
                                        nc.tensor.matmul(
                                            ps2[s][:],
                                            geluT[:, ft,
                                                  tok * P:(tok + 1) * P],
                                            w2[:, i, :],
                                            start=False,
                                            stop=(ft == FT - 1),
                                            skip_group_check=True)
                            for s in range(2):
                                t_global = qc * 4 + sp * 2 + s
                                tmp = work.tile([P, 512], BF16, tag="ftmp",
                                                bufs=1)
                                nc.vector.tensor_tensor(
                                    out=tmp[:], in0=ps2[s][:],
                                    in1=g2bc[:, fp * 512:(fp + 1) * 512],
                                    op=OP.mult)
                                oth = work.tile([P, 512], F32, tag="ot",
                                                bufs=2,
                                                name=f"ot{qc}{fp}{sp}{s}")
                                nc.gpsimd.tensor_add(
                                    oth[:],
                                    x_sb[:, t_global,
                                         fp * 512:(fp + 1) * 512],
                                    tmp[:])
                                nc.sync.dma_start(
                                    out_v[:, t_global,
                                          fp * 512:(fp + 1) * 512],
                                    oth[:])
                        yield blk

            def fc2_tail_blocks(qc):
                # post-attn: stream each w2 chunk ONCE for all 4 token
                # blocks (4 live psum banks) -- halves fc2 HBM traffic
                for fp in range(2):
                    def blk(fp=fp, qc=qc):
                        tl = fsd["tail"]
                        ps4 = [tl.tile([P, 512], F32, tag=f"tl{i}", bufs=1,
                                       name=f"ptl{i}") for i in range(4)]
                        for s in range(4):
                            nc.tensor.matmul(
                                ps4[s][:], onesrow[:, 0:P],
                                f2bs[:, fp * 512:(fp + 1) * 512],
                                start=True, stop=False)
                        for fpr in range(FT // 2):
                            w2 = fs_tile([P, 2, 512], BF16, tag="w2",
                                         bufs=3, name="w2t")
                            nc.sync.dma_start(
                                w2[:], fc2w_d[:, 2 * fpr:2 * fpr + 2,
                                              fp * 512:(fp + 1) * 512])
                            for i in range(2):
                                ft = 2 * fpr + i
                                for s in range(4):
                                    nc.tensor.matmul(
                                        ps4[s][:],
                                        geluT[:, ft, s * P:(s + 1) * P],
                                        w2[:, i, :],
                                        start=False, stop=(ft == FT - 1),
                                        skip_group_check=True)
                        for s in range(4):
                            t_global = qc * 4 + s
                            tmp = work.tile([P, 512], BF16, tag="ftmp",
                                            bufs=1)
                            nc.vector.tensor_tensor(
                                out=tmp[:], in0=ps4[s][:],
                                in1=g2bc[:, fp * 512:(fp + 1) * 512],
                                op=OP.mult)
                            oth = work.tile([P, 512], F32, tag="ot",
                                            bufs=2, name=f"otl{qc}{fp}{s}")
                            nc.gpsimd.tensor_add(
                                oth[:],
                                x_sb[:, t_global, fp * 512:(fp + 1) * 512],
                                tmp[:])
                            nc.sync.dma_start(
                                out_v[:, t_global, fp * 512:(fp + 1) * 512],
                                oth[:])
                    yield blk

            closed = {}

            def run_qc(qc, pending, producers=None):
                pend_i = 0
                with nc.named_scope(f"attn{qc}"):
                    aT = aTp.tile([P, KT, QC], BF16, tag="aT",
                                  name=f"aT_{qc}")
                    atts = []
                    for j in range(KT):
                        if producers:
                            for fn in producers.pop(j, []):
                                fn()
                        atts.append(scores_exp(qc, j))
                        for _ in range(2):
                            if pend_i < len(pending):
                                pending[pend_i]()
                                pend_i += 1
                        if j >= 1:
                            av_norm(j - 1, *atts[j - 1], aT)
                    av_norm(KT - 1, *atts[KT - 1], aT)
                if qc == 0 and not closed:
                    # h1T / V weights are dead: free prologue SBUF
                    pro_d["qs_cm"].__exit__(None, None, None)
                    pro_d["pro_cm"].__exit__(None, None, None)
                    fsd["cm"] = tc.tile_pool(name="fc_stream", bufs=2)
                    fsd["fs"] = fsd["cm"].__enter__()
                    closed["done"] = True
                if qc == 1:
                    # scores/av psum dead: free 5 banks for the 4-bank tail
                    attn_ps["psav_cm"].__exit__(None, None, None)
                    attn_ps["pss_cm"].__exit__(None, None, None)
                    fsd["tail_cm"] = tc.tile_pool(name="ps_tail", bufs=1,
                                                  space="PSUM")
                    fsd["tail"] = fsd["tail_cm"].__enter__()
                with nc.named_scope(f"proj{qc}"):
                    mv2 = work.tile([P, 4, 2], F32, tag="mv4", name="mv4")
                    for s in range(4):
                        proj_block(qc, aT, s)
                        # ln2 stats for this tile right away (fills the
                        # proj->ln2 dependency valley)
                        stats = work.tile([P, 2, 6], F32, tag="stats")
                        x_ap = x_sb[:, qc * 4 + s, :]
                        for sg in range(2):
                            nc.vector.bn_stats(
                                stats[:, sg, :],
                                x_ap[:, sg * 512:(sg + 1) * 512])
                        nc.vector.bn_aggr(mv2[:, s, :], stats[:])
                with nc.named_scope(f"ln2_{qc}"):
                    rstd2 = work.tile([P, 4], F32, tag="rstd4",
                                      name="rstd4")
                    nc.scalar.activation(rstd2[:], mv2[:, :, 1], AF.Sqrt,
                                         bias=eps_sb[:])
                    nc.vector.reciprocal(rstd2[:], rstd2[:])
                    with tc.tile_pool(name="ps_tr2", bufs=1,
                                      space="PSUM") as pstr2:
                        build_hT(h2T, eff2s, eff2h, mv2, rstd2, 0, 0,
                                 qc * 4, pstr2)
                with nc.named_scope(f"fc1_{qc}"):
                    fc1_gelu(qc)
                    while pend_i < len(pending):
                        pending[pend_i]()
                        pend_i += 1
                if qc == 1:
                    return list(fc2_tail_blocks(qc))
                return list(fc2_blocks(qc))

            producers0 = {
                0: [lambda: qk_pair(0, 0), lambda: qk_pair(1, 0),
                    lambda: load_v(0)],
                1: [lambda: qk_pair(0, 2), lambda: qk_pair(1, 2)],
                2: [lambda: qk_pair(0, 4), lambda: qk_pair(1, 4),
                    lambda: load_v(1)],
                3: [lambda: qk_pair(0, 6), lambda: qk_pair(1, 6)],
            }
            pending = run_qc(0, [], producers0)
            pending = run_qc(1, pending)
            with nc.named_scope("mlp_tail"):
                for blk in pending:
                    blk()
            fsd["tail_cm"].__exit__(None, None, None)
            fsd["cm"].__exit__(None, None, None)

        _work_cm.__exit__(None, None, None)

    nc.compile()
    return nc, names


def _get_compiled():
    if "nc" not in _CACHE:
        _CACHE["nc"], _CACHE["names"] = _build()
    return _CACHE["nc"], _CACHE["names"]


def _q8col(w, smax=224.0):
    w = np.asarray(w, np.float32)
    am = np.abs(w).max(axis=0, keepdims=True)
    s = np.where(am > 0, smax / np.maximum(am, 1e-30), 1.0)
    w8 = np.clip(w * s, -240, 240).astype(F8NP)
    return w8, s[0]


def _pmajor(w):
    w = np.asarray(w)
    kp, n = w.shape
    return np.ascontiguousarray(w.reshape(kp // P, P, n).transpose(1, 0, 2))


def _prep_maps(names, x, c, ln1_w, ln1_b, ln2_w, ln2_b, ada_w, ada_b,
               qkv_w, qkv_b, proj_w, proj_b, fc1_w, fc1_b, fc2_w, fc2_b):
    x = np.asarray(x, np.float32)
    c = np.asarray(c, np.float32)
    qkv8, s_qkv = _q8col(qkv_w)
    proj8, s_proj = _q8col(proj_w)
    qkv_b = np.asarray(qkv_b, np.float32)
    s_wv = s_qkv[2 * D:]

    def tcols(v):
        return np.asarray(v, np.float32).reshape(KT, P).T
    lnT = np.concatenate([tcols(ln1_w), tcols(ln1_b),
                          tcols(ln2_w), tcols(ln2_b)], axis=1)

    common = {
        names["lnT"]: np.ascontiguousarray(lnT, np.float32),
        names["ada_w"]: _pmajor(ada_w).astype(BF),
        names["ada_b"]: np.asarray(ada_b).astype(BF).reshape(1, -1),
        names["qkv_w"]: _pmajor(qkv8),
        names["bqk"]: np.ascontiguousarray(
            (qkv_b[0:2 * D] * S_QK).reshape(16, P).T.astype(np.float32)),
        names["dqqk"]: np.ascontiguousarray(
            (S_QK / s_qkv[0:2 * D]).reshape(16, P).T.astype(np.float32)),
        names["vbs"]: (qkv_b[2 * D:] * s_wv).astype(BF).reshape(1, D),
        names["dqv"]: (S_V / s_wv).astype(BF).reshape(1, D),
        names["proj_w"]: _pmajor(proj8),
        names["pbs"]: (np.asarray(proj_b, np.float32) * S_V * s_proj)
        .astype(BF).reshape(1, D),
        names["dqp"]: (1.0 / (S_V * s_proj)).astype(BF).reshape(1, D),
        names["fc1_w"]: _pmajor(fc1_w).astype(BF),
        names["bf1T"]: np.ascontiguousarray(
            np.asarray(fc1_b, np.float32).reshape(FT, P).T),
        names["fc2_w"]: _pmajor(fc2_w).astype(BF),
        names["f2bs"]: np.asarray(fc2_b).astype(BF).reshape(1, D),
    }
    in_maps = []
    for b in range(B):
        m = dict(common)
        m[names["x"]] = np.ascontiguousarray(
            x[b].reshape(TT, P, D).transpose(1, 0, 2)).astype(BF)
        m[names["c"]] = np.ascontiguousarray(c[b].reshape(KT, P).T)
        in_maps.append(m)
    return in_maps


def kernel(x, c, ln1_w, ln1_b, ln2_w, ln2_b, ada_w, ada_b,
           qkv_w, qkv_b, proj_w, proj_b, fc1_w, fc1_b, fc2_w, fc2_b,
           _trace=False):
    nc, names = _get_compiled()
    in_maps = _prep_maps(names, x, c, ln1_w, ln1_b, ln2_w, ln2_b,
                         ada_w, ada_b, qkv_w, qkv_b, proj_w, proj_b,
                         fc1_w, fc1_b, fc2_w, fc2_b)
    res = bass_utils.run_bass_kernel_spmd(nc, in_maps, core_ids=list(range(B)),
                                          trace=_trace)
    out = np.stack([res.results[b][names["out"]] for b in range(B)])
    if _trace:
        _CACHE["last_result"] = res
    return out


# revision 53
# speedup vs baseline: 1.2300x; 1.0226x over previous
"""DiT block kernel for 8 Trainium2 NeuronCores.

Sharding: data-parallel over batch (B=8 -> one batch element per core).

Vs the 856us baseline:
  - QKV + attn@V run fp8e4 DoubleRow (2 k-tiles per pass, 0.5 cyc/row):
    h1T cast fp8 (scale 1), qkv_w fp8 per-column, v_sb fp8 (xS_V),
    attn probs fp8 (as before). MLP + ada stay bf16 (fp8 there blows
    the 2e-2 error budget; measured by numpy ablation).
  - exp batched over two PSUM banks ([128,1024] per ACT instruction)
    to amortize the ~175ns fixed ACT overhead; halves instruction count.
  - fc1 bias moved into the gelu activation (per-partition bias AP);
    kills 64 ones-row bias matmuls.
  - small row copies / residual adds moved to the idle GPSIMD engine
    to unload DVE (drow eviction, v eviction, proj/fc2 residual adds).
  - x kept bf16 in SBUF (halves LN/norm DVE traffic and x DMA).
"""

import sys

sys.path.insert(0, "/opt/trn_rl_repo")

import numpy as np
import ml_dtypes

import concourse.bacc as bacc
import concourse.tile as tile
from concourse import mybir
from concourse import bass_utils
from concourse.masks import make_identity

F32 = mybir.dt.float32
BF16 = mybir.dt.bfloat16
FP8 = mybir.dt.float8e4
BF = ml_dtypes.bfloat16
F8NP = ml_dtypes.float8_e4m3
OP = mybir.AluOpType
AF = mybir.ActivationFunctionType
DR = mybir.MatmulPerfMode.DoubleRow

B = 8
L = 1024
D = 1024
H = 16
HD = 64
FF = 4096
LN_EPS = 1e-5
P = 128
TT = L // P
KT = D // P
FT = FF // P
QC = 512
NQC = L // QC

S_QK = 32.0          # q/k fp8 scale
S_V = 32.0           # v fp8 scale
ESC = 0.125 / (S_QK * S_QK)

_CACHE = {}


def _build():
    nc = bacc.Bacc(None, target_bir_lowering=False)
    names = {}
    with tile.TileContext(nc) as tc, \
            tc.tile_pool(name="dram", bufs=1, space="DRAM") as dram, \
            tc.tile_pool(name="per", bufs=1) as per:

        x_d = dram.tile([P, TT, D], BF16, kind="ExternalInput", name="x")
        c_d = dram.tile([P, KT], F32, kind="ExternalInput", name="c")
        lnT_d = dram.tile([P, 4 * KT], F32, kind="ExternalInput", name="lnT")
        adaw_d = dram.tile([P, KT, 6 * D], BF16, kind="ExternalInput",
                           name="ada_w")
        adab_d = dram.tile([1, 6 * D], BF16, kind="ExternalInput",
                           name="ada_b")
        qkvw_d = dram.tile([P, KT, 3 * D], FP8, kind="ExternalInput",
                           name="qkv_w")
        bqk_d = dram.tile([P, 16], F32, kind="ExternalInput", name="bqk")
        dqqk_d = dram.tile([P, 16], F32, kind="ExternalInput", name="dqqk")
        vbs_d = dram.tile([1, D], BF16, kind="ExternalInput", name="vbs")
        dqv_d = dram.tile([1, D], BF16, kind="ExternalInput", name="dqv")
        projw_d = dram.tile([P, KT, D], FP8, kind="ExternalInput",
                            name="proj_w")
        pbs_d = dram.tile([1, D], BF16, kind="ExternalInput", name="pbs")
        dqp_d = dram.tile([1, D], BF16, kind="ExternalInput", name="dqp")
        fc1w_d = dram.tile([P, KT, FF], BF16, kind="ExternalInput",
                           name="fc1_w")
        bf1T_d = dram.tile([P, FT], F32, kind="ExternalInput", name="bf1T")
        fc2w_d = dram.tile([P, FT, D], BF16, kind="ExternalInput",
                           name="fc2_w")
        f2bs_d = dram.tile([1, D], BF16, kind="ExternalInput", name="f2bs")
        out_d = dram.tile([L, D], F32, kind="ExternalOutput", name="out")
        for t, n in [(x_d, "x"), (c_d, "c"), (lnT_d, "lnT"),
                     (adaw_d, "ada_w"), (adab_d, "ada_b"),
                     (qkvw_d, "qkv_w"), (bqk_d, "bqk"), (dqqk_d, "dqqk"),
                     (vbs_d, "vbs"), (dqv_d, "dqv"),
                     (projw_d, "proj_w"), (pbs_d, "pbs"), (dqp_d, "dqp"),
                     (fc1w_d, "fc1_w"), (bf1T_d, "bf1T"),
                     (fc2w_d, "fc2_w"), (f2bs_d, "f2bs"), (out_d, "out")]:
            names[n] = t.name
        out_v = out_d[:].rearrange("(t p) d -> p t d", p=P)

        x_sb = per.tile([P, TT, D], BF16)
        for t in range(TT):
            nc.sync.dma_start(x_sb[:, t, :], x_d[:, t, :])
        ident = per.tile([P, P], BF16)
        make_identity(nc, ident[:])
        onesrow = per.tile([1, 512], BF16)
        nc.vector.memset(onesrow[:], 1.0)
        eps_sb = per.tile([P, 1], F32)
        nc.vector.memset(eps_sb[:], LN_EPS)

        lnT = per.tile([P, 4 * KT], F32)
        nc.sync.dma_start(lnT[:], lnT_d[:])
        bqk = per.tile([P, 16], F32)
        nc.sync.dma_start(bqk[:], bqk_d[:])
        dqqk = per.tile([P, 16], F32)
        nc.sync.dma_start(dqqk[:], dqqk_d[:])
        vbs = per.tile([1, D], BF16)
        nc.sync.dma_start(vbs[:], vbs_d[:])
        dqv_row = per.tile([1, D], BF16)
        nc.sync.dma_start(dqv_row[:], dqv_d[:])
        pbs = per.tile([1, D], BF16)
        nc.sync.dma_start(pbs[:], pbs_d[:])
        dqp_row = per.tile([1, D], BF16)
        nc.sync.dma_start(dqp_row[:], dqp_d[:])

        f2bs = per.tile([1, D], BF16)
        nc.sync.dma_start(f2bs[:], f2bs_d[:])
        bf1T = per.tile([P, FT], F32)
        nc.sync.dma_start(bf1T[:], bf1T_d[:])
        adab = per.tile([1, 6 * D], BF16)
        nc.sync.dma_start(adab[:], adab_d[:])
        projw = per.tile([P, KT, D], FP8)
        for k in range(KT):
            nc.sync.dma_start(projw[:, k, :], projw_d[:, k, :])
        dqvb = per.tile([P, D], BF16)
        nc.gpsimd.partition_broadcast(dqvb[:], dqv_row[:])

        eff1s = per.tile([P, KT], F32)
        eff1h = per.tile([P, KT], F32)
        eff2s = per.tile([P, KT], F32)
        eff2h = per.tile([P, KT], F32)
        g1bc = per.tile([P, D], BF16)
        g2bc = per.tile([P, D], BF16)

        q8 = per.tile([P, KT, L], FP8)
        k8 = per.tile([P, KT, L], FP8)
        v_sb = per.tile([P, TT, H, HD + 1], FP8)
        nc.vector.memset(v_sb[:, :, :, HD:HD + 1], 1.0)
        h2T = per.tile([P, KT, QC], BF16)
        geluT = per.tile([P, FT, QC], BF16)

        _work_cm = tc.tile_pool(name="work", bufs=2)
        work = _work_cm.__enter__()

        def ln_stats(x_aps):
            n = len(x_aps)
            mv = work.tile([P, n, 2], F32, tag=f"mv{n}", name=f"mv{n}")
            for i, x_ap in enumerate(x_aps):
                stats = work.tile([P, 2, 6], F32, tag="stats")
                for sg in range(2):
                    nc.vector.bn_stats(stats[:, sg, :],
                                       x_ap[:, sg * 512:(sg + 1) * 512])
                nc.vector.bn_aggr(mv[:, i, :], stats[:])
            rstd = work.tile([P, n], F32, tag=f"rstd{n}", name=f"rstd{n}")
            nc.scalar.activation(rstd[:], mv[:, :, 1], AF.Sqrt, bias=eps_sb[:])
            nc.vector.reciprocal(rstd[:], rstd[:])
            return mv, rstd

        mv1, rstd1 = ln_stats([x_sb[:, t, :] for t in range(TT)])

        # ---------------- ada (bf16, 512-wide moving weights) -----------
        with nc.named_scope("ada"), \
                tc.tile_pool(name="ada_sb", bufs=1) as asb, \
                tc.tile_pool(name="ada_stream", bufs=2) as ast, \
                tc.tile_pool(name="ps_ada", bufs=2, space="PSUM") as psa, \
                tc.tile_pool(name="ps_t6", bufs=1, space="PSUM") as pst6:
            c_sb = asb.tile([P, KT], F32)
            nc.sync.dma_start(c_sb[:], c_d[:])
            silu_b = asb.tile([P, KT], BF16)
            nc.scalar.activation(silu_b[:], c_sb[:], AF.Silu)
            ssgb = asb.tile([1, 6 * D], BF16, name="ssgb")

            def ada_chunk(nch, eng=None):
                aw = ast.tile([P, KT, 512], BF16, tag="aw")
                (eng or nc.sync).dma_start(
                    aw[:], adaw_d[:, :, nch * 512:(nch + 1) * 512])
                pa = psa.tile([1, 512], F32, tag="ada")
                nc.tensor.matmul(pa[:], onesrow[:, 0:1],
                                 adab[:, nch * 512:(nch + 1) * 512],
                                 start=True, stop=False)
                for k in range(KT):
                    nc.tensor.matmul(pa[:], silu_b[:, k:k + 1], aw[:, k, :],
                                     start=False, stop=(k == KT - 1),
                                     skip_group_check=True)
                nc.vector.tensor_copy(
                    ssgb[:, nch * 512:(nch + 1) * 512], pa[:])

            # shift/scale rows -> feature-major cols; v' = sh1,sc1,sh2,sc2
            pt6 = pst6.tile([P, 32, 2], BF16)

            def ada_tr(vi, v):
                for ch in range(KT):
                    i = ch * 4 + vi
                    nc.tensor.transpose(
                        pt6[:, i, 0:1],
                        ssgb[0:1, v * D + ch * P:v * D + (ch + 1) * P],
                        ident[0:1, 0:1])
                nc.vector.tensor_copy(ssgT[:, :, vi], pt6[:, vi:32:4, 0])

            ssgT = asb.tile([P, KT, 4], F32, name="ssgT")
            t0 = asb.tile([P, KT], F32, name="t0")
            t1 = asb.tile([P, KT], F32, name="t1")
            # chunk order puts the attn shift/scale (-> h1T -> qkv) first
            for nch in (0, 1, 2, 3):
                ada_chunk(nch)
            ada_tr(0, 0)
            ada_tr(1, 1)
            nc.vector.tensor_scalar_add(t0[:], ssgT[:, :, 1], 1.0)
            nc.vector.tensor_mul(eff1s[:], t0[:], lnT[:, 0:KT])
            nc.vector.tensor_mul(t1[:], t0[:], lnT[:, KT:2 * KT])
            nc.vector.tensor_add(eff1h[:], t1[:], ssgT[:, :, 0])
            for nch in (6, 7, 8, 9):
                ada_chunk(nch)
            ada_tr(2, 3)
            ada_tr(3, 4)
            nc.vector.tensor_scalar_add(t0[:], ssgT[:, :, 3], 1.0)
            nc.vector.tensor_mul(eff2s[:], t0[:], lnT[:, 2 * KT:3 * KT])
            nc.vector.tensor_mul(t1[:], t0[:], lnT[:, 3 * KT:4 * KT])
            nc.vector.tensor_add(eff2h[:], t1[:], ssgT[:, :, 2])
            for nch in (4, 5, 10, 11):
                ada_chunk(nch)
            g1row = asb.tile([1, D], BF16, name="g1row")
            nc.vector.tensor_mul(g1row[:], ssgb[:, 2 * D:3 * D], dqp_row[:])
            nc.gpsimd.partition_broadcast(g1bc[:], g1row[:])
            nc.gpsimd.partition_broadcast(g2bc[:], ssgb[:, 5 * D:6 * D])

        # ---------------- LN -> hT (transpose + fused modulate) ---------
        def build_hT(dst, eff_s, eff_h, mv, rstd, idx0, tg, toff, pstr):
            norms = []
            for s in range(4):
                t = toff + s
                nrm = work.tile([P, D], BF16, tag="nrm", bufs=4)
                i = idx0 + s
                nc.vector.tensor_scalar(out=nrm[:], in0=x_sb[:, t, :],
                                        scalar1=mv[:, i, 0:1],
                                        scalar2=rstd[:, i:i + 1],
                                        op0=OP.subtract, op1=OP.mult)
                norms.append(nrm)
            for k in range(KT):
                pt = pstr.tile([P, 4 * P], BF16, tag="tr")
                for s in range(4):
                    nc.tensor.transpose(pt[:, s * P:(s + 1) * P],
                                        norms[s][:, k * P:(k + 1) * P],
                                        ident[:])
                nc.vector.tensor_scalar(
                    out=dst[:, k, tg * 512:(tg + 1) * 512], in0=pt[:],
                    scalar1=eff_s[:, k:k + 1], scalar2=eff_h[:, k:k + 1],
                    op0=OP.mult, op1=OP.add)

        # ---------------- prologue: h1T + QKV (fp8 DoubleRow) -----------
        pro_d = {}
        wv_half = []

        def open_prologue():
            pro_d["pro_cm"] = tc.tile_pool(name="pro", bufs=1)
            pro = pro_d["pro_cm"].__enter__()
            h1T = pro.tile([P, KT, L], FP8, name="h1T")
            with tc.tile_pool(name="ps_tr1", bufs=1, space="PSUM") as pstr1:
                for tg in range(2):
                    build_hT(h1T, eff1s, eff1h, mv1, rstd1, tg * 4, tg,
                             tg * 4, pstr1)
            pro_d["qs_cm"] = tc.tile_pool(name="qkv_stream", bufs=3)
            pro_d["qst"] = pro_d["qs_cm"].__enter__()
            pro_d["h1T"] = h1T

        def qk_pair(qk, jp):
            fbase = D if qk == 0 else 0
            dst = k8 if qk == 0 else q8
            wj = pro_d["qst"].tile([P, KT, 2 * P], FP8, tag="wj", bufs=2,
                                   name="wjt")
            nc.sync.dma_start(
                wj[:], qkvw_d[:, :, fbase + jp * P:fbase + (jp + 2) * P])
            for jo in range(2):
                jj = jp + jo
                ci = (fbase + jj * P) // P
                for tg in range(2):
                    pq = mm_tile(jj * 2 + tg, "pq")
                    for k in range(0, KT, 2):
                        nc.tensor.matmul(
                            pq[:], wj[:, k:k + 2, jo * P:(jo + 1) * P],
                            pro_d["h1T"][:, k:k + 2,
                                         tg * 512:(tg + 1) * 512],
                            start=(k == 0), stop=(k == KT - 2),
                            perf_mode=DR)
                    nc.vector.tensor_scalar(
                        out=dst[:, jj, tg * 512:(tg + 1) * 512], in0=pq[:],
                        scalar1=dqqk[:, ci:ci + 1], scalar2=bqk[:, ci:ci + 1],
                        op0=OP.mult, op1=OP.add)

        def v_chunk(tt, fh):
            pv = mm_tile(tt * 2 + fh, "pv")
            nc.tensor.matmul(pv[:], onesrow[:, 0:P],
                             vbs[:, fh * 512:(fh + 1) * 512],
                             start=True, stop=False)
            for k in range(0, KT, 2):
                nc.tensor.matmul(pv[:],
                                 pro_d["h1T"][:, k:k + 2, tt * P:(tt + 1) * P],
                                 wv_half[fh][:, k:k + 2, :],
                                 start=False, stop=(k == KT - 2),
                                 skip_group_check=True, perf_mode=DR)
            nc.vector.tensor_tensor(
                out=v_sb[:, tt, fh * 8:(fh + 1) * 8, 0:HD], in0=pv[:],
                in1=dqvb[:, fh * 512:(fh + 1) * 512], op=OP.mult)

        def load_wv(fh):
            wv = pro_d["qst"].tile([P, KT, 512], FP8, tag="wv", bufs=1,
                          name=f"wv{fh}")
            nc.sync.dma_start(wv[:],
                              qkvw_d[:, :, 2 * D + fh * 512:
                                     2 * D + (fh + 1) * 512])
            wv_half.append(wv)

        # ---------------- attention + MLP, phase-pipelined --------------
        fsd = {}

        def fs_tile(*a, **k):
            return fsd["fs"].tile(*a, **k)

        with tc.tile_pool(name="attn", bufs=2) as ap, \
                tc.tile_pool(name="aTp", bufs=1) as aTp, \
                tc.tile_pool(name="ps_m", bufs=1, space="PSUM") as psm:

            attn_ps = {}
            attn_ps["pss_cm"] = tc.tile_pool(name="ps_s", bufs=2,
                                             space="PSUM")
            pss = attn_ps["pss_cm"].__enter__()
            attn_ps["psav_cm"] = tc.tile_pool(name="ps_av", bufs=1,
                                              space="PSUM")
            psav = attn_ps["psav_cm"].__enter__()

            def mm_tile(i, nm):
                return psm.tile([P, 512], F32, tag=f"f2_{i % 2}", bufs=1,
                                name=nm)

            open_prologue()

            def load_v(fh):
                load_wv(fh)
                for tt in range(TT):
                    v_chunk(tt, fh)

            def scores_exp(qc, j):
                q0 = qc * QC
                attA = ap.tile([P, TT, QC], FP8, tag="attA")
                attB = ap.tile([P, TT, QC], FP8, tag="attB")
                for hh, att in ((0, attA), (1, attB)):
                    p0 = hh * HD
                    for mp in range(TT // 2):
                        ps2 = pss.tile([P, 2, QC], F32, tag="sc", name="ps2")
                        for s in range(2):
                            m = 2 * mp + s
                            nc.tensor.matmul(
                                ps2[:, s, :],
                                k8[p0:p0 + HD, j, m * P:(m + 1) * P],
                                q8[p0:p0 + HD, j, q0:q0 + QC],
                                start=True, stop=True,
                                tile_position=(p0, 0))
                        nc.scalar.activation(att[:, 2 * mp:2 * mp + 2, :],
                                             ps2[:], AF.Exp, scale=ESC)
                return attA, attB

            def av_norm(j, attA, attB, aT):
                for hh, att in ((0, attA), (1, attB)):
                    h = 2 * j + hh
                    pu = psav.tile([HD + 1, QC], F32, tag="pu")
                    for mp in range(TT // 2):
                        nc.tensor.matmul(
                            pu[:], v_sb[:, 2 * mp:2 * mp + 2, h, :],
                            att[:, 2 * mp:2 * mp + 2, :],
                            start=(mp == 0), stop=(mp == TT // 2 - 1),
                            perf_mode=DR)
                    drow = work.tile([1, QC], F32, tag="drow", bufs=1)
                    nc.vector.tensor_copy(drow[:], pu[HD:HD + 1, :])
                    dbc = work.tile([HD, QC], F32, tag="dbc", bufs=1)
                    nc.gpsimd.partition_broadcast(dbc[:], drow[:])
                    rec = work.tile([HD, QC], F32, tag="rec", bufs=1)
                    nc.vector.reciprocal_approx_fast(rec[:], dbc[:])
                    nc.vector.tensor_tensor(
                        out=aT[hh * 64:(hh + 1) * 64, j, :],
                        in0=pu[0:HD, :], in1=rec[:], op=OP.mult)

            def proj_block(qc, aT, s):
                t_global = qc * 4 + s
                for fh in range(2):
                    pp = mm_tile(s * 2 + fh, "pp")
                    nc.tensor.matmul(pp[:], onesrow[:, 0:P],
                                     pbs[:, fh * 512:(fh + 1) * 512],
                                     start=True, stop=False)
                    for k in range(KT):
                        nc.tensor.matmul(
                            pp[:], aT[:, k, s * P:(s + 1) * P],
                            projw[:, k, fh * 512:(fh + 1) * 512],
                            start=False, stop=(k == KT - 1),
                            skip_group_check=True)
                    tmp = work.tile([P, 512], BF16, tag="ptmp", bufs=1)
                    nc.vector.tensor_tensor(
                        out=tmp[:], in0=pp[:],
                        in1=g1bc[:, fh * 512:(fh + 1) * 512], op=OP.mult)
                    nc.gpsimd.tensor_add(
                        x_sb[:, t_global, fh * 512:(fh + 1) * 512],
                        x_sb[:, t_global, fh * 512:(fh + 1) * 512],
                        tmp[:])

            def fc1_gelu(qc):
                for fcp in range(0, FT, 4):
                    w1 = fs_tile([P, KT, 4 * P], BF16, tag="w1", bufs=3,
                                 name="w1t")
                    nc.sync.dma_start(w1[:],
                                      fc1w_d[:, :, fcp * P:(fcp + 4) * P])
                    for i in range(4):
                        fc = fcp + i
                        pf = mm_tile(fc, "pf")
                        for k in range(KT):
                            nc.tensor.matmul(pf[:],
                                             w1[:, k, i * P:(i + 1) * P],
                                             h2T[:, k, :],
                                             start=(k == 0),
                                             stop=(k == KT - 1))
                        nc.scalar.activation(geluT[:, fc, :], pf[:],
                                             AF.Gelu_apprx_tanh,
                                             bias=bf1T[:, fc:fc + 1])

            def fc2_blocks(qc):
                # s-pairs share the streamed w2 chunk; 2 psum banks live
                for sp in range(2):
                    for fp in range(2):
                        def blk(sp=sp, fp=fp, qc=qc):
                            ps2 = [mm_tile(0, "pf2a"), mm_tile(1, "pf2b")]
                            for s in range(2):
                                nc.tensor.matmul(
                                    ps2[s][:], onesrow[:, 0:P],
                                    f2bs[:, fp * 512:(fp + 1) * 512],
                                    start=True, stop=False)
                            for fpr in range(FT // 2):
                                w2 = fs_tile([P, 2, 512], BF16, tag="w2",
                                             bufs=3, name="w2t")
                                nc.sync.dma_start(
                                    w2[:], fc2w_d[:, 2 * fpr:2 * fpr + 2,
                                                  fp * 512:(fp + 1) * 512])
                                for i in range(2):
                                    ft = 2 * fpr + i
                                    for s in range(2):
                                        tok = sp * 2 + s
                                        nc.tensor.matmul(
                                            ps2[s][:],
                                            geluT[:, ft,
                                                  tok * P:(tok + 1) * P],
                                            w2[:, i, :],
                                            start=False,
                                            stop=(ft == FT - 1),
                                            skip_group_check=True)
                            for s in range(2):
                                t_global = qc * 4 + sp * 2 + s
                                tmp = work.tile([P, 512], BF16, tag="ftmp",
                                                bufs=1)
                                nc.vector.tensor_tensor(
                                    out=tmp[:], in0=ps2[s][:],
                                    in1=g2bc[:, fp * 512:(fp + 1) * 512],
                                    op=OP.mult)
                                oth = work.tile([P, 512], F32, tag="ot",
                                                bufs=2,
                                                name=f"ot{qc}{fp}{sp}{s}")
                                nc.gpsimd.tensor_add(
                                    oth[:],
                                    x_sb[:, t_global,
                                         fp * 512:(fp + 1) * 512],
                                    tmp[:])
                                nc.sync.dma_start(
                                    out_v[:, t_global,
                                          fp * 512:(fp + 1) * 512],
                                    oth[:])
                        yield blk

            def fc2_tail_blocks(qc):
                # post-attn: stream each w2 chunk ONCE for all 4 token
                # blocks (4 live psum banks) -- halves fc2 HBM traffic
                for fp in range(2):
                    def blk(fp=fp, qc=qc):
                        tl = fsd["tail"]
                        ps4 = [tl.tile([P, 512], F32, tag=f"tl{i}", bufs=1,
                                       name=f"ptl{i}") for i in range(4)]
                        for s in range(4):
                            nc.tensor.matmul(
                                ps4[s][:], onesrow[:, 0:P],
                                f2bs[:, fp * 512:(fp + 1) * 512],
                                start=True, stop=False)
                        for fpr in range(FT // 2):
                            w2 = fs_tile([P, 2, 512], BF16, tag="w2",
                                         bufs=3, name="w2t")
                            nc.sync.dma_start(
                                w2[:], fc2w_d[:, 2 * fpr:2 * fpr + 2,
                                              fp * 512:(fp + 1) * 512])
                            for i in range(2):
                                ft = 2 * fpr + i
                                for s in range(4):
                                    nc.tensor.matmul(
                                        ps4[s][:],
                                        geluT[:, ft, s * P:(s + 1) * P],
                                        w2[:, i, :],
                                        start=False, stop=(ft == FT - 1),
                                        skip_group_check=True)
                        for s in range(4):
                            t_global = qc * 4 + s
                            tmp = work.tile([P, 512], BF16, tag="ftmp",
                                            bufs=1)
                            nc.vector.tensor_tensor(
                                out=tmp[:], in0=ps4[s][:],
                                in1=g2bc[:, fp * 512:(fp + 1) * 512],
                                op=OP.mult)
                            oth = work.tile([P, 512], F32, tag="ot",
                                            bufs=2, name=f"otl{qc}{fp}{s}")
                            nc.gpsimd.tensor_add(
                                oth[:],
                                x_sb[:, t_global, fp * 512:(fp + 1) * 512],
                                tmp[:])
                            nc.sync.dma_start(
                                out_v[:, t_global, fp * 512:(fp + 1) * 512],
                                oth[:])
                    yield blk

            closed = {}

            def run_qc(qc, pending, producers=None):
                pend_i = 0
                with nc.named_scope(f"attn{qc}"):
                    aT = aTp.tile([P, KT, QC], BF16, tag="aT",
                                  name=f"aT_{qc}")
                    atts = []
                    for j in range(KT):
                        if producers:
                            for fn in producers.pop(j, []):
                                fn()
                        atts.append(scores_exp(qc, j))
                        for _ in range(2):
                            if pend_i < len(pending):
                                pending[pend_i]()
                                pend_i += 1
                        if j >= 1:
                            av_norm(j - 1, *atts[j - 1], aT)
                    av_norm(KT - 1, *atts[KT - 1], aT)
                if qc == 0 and not closed:
                    # h1T / V weights are dead: free prologue SBUF
                    pro_d["qs_cm"].__exit__(None, None, None)
                    pro_d["pro_cm"].__exit__(None, None, None)
                    fsd["cm"] = tc.tile_pool(name="fc_stream", bufs=2)
                    fsd["fs"] = fsd["cm"].__enter__()
                    closed["done"] = True
                if qc == 1:
                    # scores/av psum dead: free 5 banks for the 4-bank tail
                    attn_ps["psav_cm"].__exit__(None, None, None)
                    attn_ps["pss_cm"].__exit__(None, None, None)
                    fsd["tail_cm"] = tc.tile_pool(name="ps_tail", bufs=1,
                                                  space="PSUM")
                    fsd["tail"] = fsd["tail_cm"].__enter__()
                with nc.named_scope(f"proj{qc}"):
                    mv2 = work.tile([P, 4, 2], F32, tag="mv4", name="mv4")
                    for s in range(4):
                        proj_block(qc, aT, s)
                        # ln2 stats for this tile right away (fills the
                        # proj->ln2 dependency valley)
                        stats = work.tile([P, 2, 6], F32, tag="stats")
                        x_ap = x_sb[:, qc * 4 + s, :]
                        for sg in range(2):
                            nc.vector.bn_stats(
                                stats[:, sg, :],
                                x_ap[:, sg * 512:(sg + 1) * 512])
                        nc.vector.bn_aggr(mv2[:, s, :], stats[:])
                with nc.named_scope(f"ln2_{qc}"):
                    rstd2 = work.tile([P, 4], F32, tag="rstd4",
                                      name="rstd4")
                    nc.scalar.activation(rstd2[:], mv2[:, :, 1], AF.Sqrt,
                                         bias=eps_sb[:])
                    nc.vector.reciprocal(rstd2[:], rstd2[:])
                    with tc.tile_pool(name="ps_tr2", bufs=1,
                                      space="PSUM") as pstr2:
                        build_hT(h2T, eff2s, eff2h, mv2, rstd2, 0, 0,
                                 qc * 4, pstr2)
                with nc.named_scope(f"fc1_{qc}"):
                    fc1_gelu(qc)
                    while pend_i < len(pending):
                        pending[pend_i]()
                        pend_i += 1
                if qc == 1:
                    return list(fc2_tail_blocks(qc))
                return list(fc2_blocks(qc))

            producers0 = {
                0: [lambda: qk_pair(0, 0), lambda: qk_pair(1, 0),
                    lambda: load_v(0)],
                1: [lambda: qk_pair(0, 2), lambda: qk_pair(1, 2)],
                2: [lambda: qk_pair(0, 4), lambda: qk_pair(1, 4),
                    lambda: load_v(1)],
                3: [lambda: qk_pair(0, 6), lambda: qk_pair(1, 6)],
            }
            pending = run_qc(0, [], producers0)
            pending = run_qc(1, pending)
            with nc.named_scope("mlp_tail"):
                for blk in pending:
                    blk()
            fsd["tail_cm"].__exit__(None, None, None)
            fsd["cm"].__exit__(None, None, None)

        _work_cm.__exit__(None, None, None)

    nc.compile()
    return nc, names


def _get_compiled():
    if "nc" not in _CACHE:
        _CACHE["nc"], _CACHE["names"] = _build()
    return _CACHE["nc"], _CACHE["names"]


def _q8col(w, smax=224.0):
    w = np.asarray(w, np.float32)
    am = np.abs(w).max(axis=0, keepdims=True)
    s = np.where(am > 0, smax / np.maximum(am, 1e-30), 1.0)
    w8 = np.clip(w * s, -240, 240).astype(F8NP)
    return w8, s[0]


def _pmajor(w):
    w = np.asarray(w)
    kp, n = w.shape
    return np.ascontiguousarray(w.reshape(kp // P, P, n).transpose(1, 0, 2))


def _prep_maps(names, x, c, ln1_w, ln1_b, ln2_w, ln2_b, ada_w, ada_b,
               qkv_w, qkv_b, proj_w, proj_b, fc1_w, fc1_b, fc2_w, fc2_b):
    x = np.asarray(x, np.float32)
    c = np.asarray(c, np.float32)
    qkv8, s_qkv = _q8col(qkv_w)
    proj8, s_proj = _q8col(proj_w)
    qkv_b = np.asarray(qkv_b, np.float32)
    s_wv = s_qkv[2 * D:]

    def tcols(v):
        return np.asarray(v, np.float32).reshape(KT, P).T
    lnT = np.concatenate([tcols(ln1_w), tcols(ln1_b),
                          tcols(ln2_w), tcols(ln2_b)], axis=1)

    common = {
        names["lnT"]: np.ascontiguousarray(lnT, np.float32),
        names["ada_w"]: _pmajor(ada_w).astype(BF),
        names["ada_b"]: np.asarray(ada_b).astype(BF).reshape(1, -1),
        names["qkv_w"]: _pmajor(qkv8),
        names["bqk"]: np.ascontiguousarray(
            (qkv_b[0:2 * D] * S_QK).reshape(16, P).T.astype(np.float32)),
        names["dqqk"]: np.ascontiguousarray(
            (S_QK / s_qkv[0:2 * D]).reshape(16, P).T.astype(np.float32)),
        names["vbs"]: (qkv_b[2 * D:] * s_wv).astype(BF).reshape(1, D),
        names["dqv"]: (S_V / s_wv).astype(BF).reshape(1, D),
        names["proj_w"]: _pmajor(proj8),
        names["pbs"]: (np.asarray(proj_b, np.float32) * S_V * s_proj)
        .astype(BF).reshape(1, D),
        names["dqp"]: (1.0 / (S_V * s_proj)).astype(BF).reshape(1, D),
        names["fc1_w"]: _pmajor(fc1_w).astype(BF),
        names["bf1T"]: np.ascontiguousarray(
            np.asarray(fc1_b, np.float32).reshape(FT, P).T),
        names["fc2_w"]: _pmajor(fc2_w).astype(BF),
        names["f2bs"]: np.asarray(fc2_b).astype(BF).reshape(1, D),
    }
    in_maps = []
    for b in range(B):
        m = dict(common)
        m[names["x"]] = np.ascontiguousarray(
            x[b].reshape(TT, P, D).transpose(1, 0, 2)).astype(BF)
        m[names["c"]] = np.ascontiguousarray(c[b].reshape(KT, P).T)
        in_maps.append(m)
    return in_maps


def kernel(x, c, ln1_w, ln1_b, ln2_w, ln2_b, ada_w, ada_b,
           qkv_w, qkv_b, proj_w, proj_b, fc1_w, fc1_b, fc2_w, fc2_b,
           _trace=False):
    nc, names = _get_compiled()
    in_maps = _prep_maps(names, x, c, ln1_w, ln1_b, ln2_w, ln2_b,
                         ada_w, ada_b, qkv_w, qkv_b, proj_w, proj_b,
                         fc1_w, fc1_b, fc2_w, fc2_b)
    res = bass_utils.run_bass_kernel_spmd(nc, in_maps, core_ids=list(range(B)),
                                          trace=_trace)
    out = np.stack([res.results[b][names["out"]] for b in range(B)])
    if _trace:
        _CACHE["last_result"] = res
    return out
